# revision 43
# baseline (speedup 1.0000x reference)
"""AdaConv (nn_AdaConv_46445776339355) — 8-core TRN2 Bass kernel.

Strategy
--------
Data-parallel over batch N=8: core n owns sample n end-to-end for the heavy
instance-norm + grouped-conv work.  The kernel *generator* (dw_w is 256 MiB)
is tensor-parallel: core j holds the output-channel shard j of dw_w / pwk_w,
computes the generated kernels for ALL samples on its shard, and an AllToAll
routes each sample's kernels to its owning core.

Algebraic fusions (all computed on device):
  * pointwise o depthwise = one fused per-group kernel  F_t = P @ W_t
  * instance norm folded into the fused kernels:
        y = sum_t F_t @ ((x-mu)/sigma)_pad = sum_t (F_t/sigma_ci) @ x_pad - B
    with B = sum_t (F_t/sigma) @ mu  (position independent, reflect-pad safe)
  * biases (dw_b, pwk_b, pwb_b) folded in via K=1 matmul rows.

The grouped conv (8 groups of 64->64 ch, 3x3) runs as 4 concurrent 64x64
matmuls in the 4 PE-array quadrants (tile_position packing) => full 128x128
PE utilization, bf16, 9 shifted-AP taps accumulating in PSUM.

v2 flow restructure (from trace analysis of the 455us baseline):
  * generator weight chunks stream first on the sync/scalar DMA queues with
    nothing queued ahead of them (the 21 MiB stream is the serial critical
    path: ~60us at full BW); pwk chunk DMAs are hoisted before the PSUM
    evacuation copies so the stream never stalls behind compute waits.
  * img (predicted) loads are deferred behind the weight stream via a
    dependency stub on a late weight chunk, in 2 half-tile DMAs each so the
    stats reductions overlap arrival.
  * instance-norm stats for pairs 0/1 run on VectorE during the generator
    phase; pairs 2/3 run on GpSimdE during conv h=0 (VectorE evicts PSUM).
  * PSUM eviction split: ScalarE takes pair A, VectorE pair B; output staged
    bf16 (halves write-back DMA; rel-err budget is loose).
  * F-build gathers collapsed to 4 strided DMAs per half.
"""

import sys
import numpy as np

sys.path.insert(0, "/opt/trn_rl_repo")

import ml_dtypes

BF16 = ml_dtypes.bfloat16

# ---------------- problem constants (hardcoded per the harness contract) ----
N = 8            # batch == number of cores
C = 512          # channels
H = W = 128
HW = H * W       # 16384
PW = W + 2       # 130 padded
PA = PW * PW     # 16900
PAH = 65 * PW    # 8450: half-tile split (pad rows 0-64 | 65-129)
SD = 512         # style dim
NG = 8           # groups
GS = 64          # group size (channels per group)
KDW = SD * 4     # 2048 contraction dim of the dw generator
OSH = 4096       # dw/pwk output-channel shard per core (32768/8); == one group block
NTAP = 9
EPS = 1e-5
VAR_CORR = float(HW) / float(HW - 1)  # ddof=1 correction

# device output channel order: per pair of groups (2h, 2h+1) natural, the odd
# pairs (pB) have their two 64-blocks swapped (quadrant output packing).
TAU_BLOCK = [0, 1, 3, 2, 4, 5, 7, 6]  # true 64-block of device 64-block d


def _host_prep(style_encoding, predicted, dw_w, dw_b, pwk_w, pwk_b, pwb_w, pwb_b):
    """Pure data-movement / dtype-cast host prep. Returns per-core input maps."""
    f32 = np.float32
    se = np.asarray(style_encoding, f32)
    pred = np.asarray(predicted, f32)

    # --- patches for the dw generator conv: reflect pad 1, 2x2 windows s=2 ---
    sep = np.pad(se, ((0, 0), (0, 0), (1, 1), (1, 1)), mode="reflect")  # (8,512,6,6)
    blocks = sep.reshape(N, SD, 3, 2, 3, 2)  # [n,c,oy,ky,ox,kx]
    patches = np.ascontiguousarray(
        blocks.transpose(1, 3, 5, 0, 2, 4).reshape(KDW, N * NTAP)
    ).astype(BF16)  # [(c,ky,kx), (n,oy,ox)] = [2048, 72]
    # pre-tiled for a contiguous SBUF DMA: [128, kt, m]
    patches_t = np.ascontiguousarray(
        patches.reshape(16, 128, N * NTAP).transpose(1, 0, 2).reshape(128, 16 * N * NTAP)
    )

    # --- dw generator weights, transposed + sharded on output channels ---
    dwt_full = np.ascontiguousarray(dw_w.reshape(C * GS, KDW).T).astype(BF16)  # [2048, 32768]
    dwb_full = np.asarray(dw_b, f32).reshape(1, C * GS).astype(BF16)

    # --- pwk: permute columns to (g, cm, co2) so the gathered row IS P^T ---
    pwk_t = np.asarray(pwk_w, f32).reshape(NG, GS, GS, SD)  # [g, co2, cm, sd]
    pwkt_full = np.ascontiguousarray(
        pwk_t.transpose(3, 0, 2, 1).reshape(SD, C * GS)
    ).astype(BF16)  # [sd, (g, cm, co2)]
    pwkb_full = (
        np.asarray(pwk_b, f32).reshape(NG, GS, GS).transpose(0, 2, 1).reshape(1, C * GS)
    ).astype(BF16)

    # --- pwb: transposed, columns in DEVICE channel order tau ---
    tau_rows = np.concatenate([np.arange(GS) + t * GS for t in TAU_BLOCK])  # [512]
    pwbt = np.ascontiguousarray(np.asarray(pwb_w, f32)[tau_rows, :].T).astype(BF16)  # [sd, out_dev]
    pwbt_t = np.ascontiguousarray(
        pwbt.reshape(4, 128, C).transpose(1, 0, 2).reshape(128, 4 * C))
    pwbb = np.asarray(pwb_b, f32)[tau_rows].reshape(1, C).astype(BF16)

    # --- styleT for sd computation on device: pre-tiled [128, kt, (n, px)] f32 ---
    styleT = np.ascontiguousarray(se.transpose(1, 0, 2, 3).reshape(SD, N * 16)).astype(f32)
    styleT_t = np.ascontiguousarray(
        styleT.reshape(4, 128, N * 16).transpose(1, 0, 2).reshape(128, 4 * N * 16))

    in_maps = []
    for j in range(N):
        pp = np.pad(pred[j], ((0, 0), (1, 1), (1, 1)), mode="reflect").reshape(C, PA)
        sel = np.zeros((128, N), f32)
        sel[:, j] = 1.0
        in_maps.append(
            dict(
                pred_pad=np.ascontiguousarray(pp).astype(BF16),
                patches=patches_t,
                styleT=styleT_t,
                sel=sel,
                dwt=np.ascontiguousarray(dwt_full[:, j * OSH:(j + 1) * OSH]),
                dwb=np.ascontiguousarray(dwb_full[:, j * OSH:(j + 1) * OSH]),
                pwkt=np.ascontiguousarray(pwkt_full[:, j * OSH:(j + 1) * OSH]),
                pwkb=np.ascontiguousarray(pwkb_full[:, j * OSH:(j + 1) * OSH]),
                pwbt=pwbt_t,
                pwbb=pwbb,
            )
        )
    return in_maps


def _unshard(results):
    """results[j]['out'] is [512, 16384] bf16 in device channel order."""
    out = np.empty((N, C, H, W), np.float32)
    for j in range(N):
        dev = np.asarray(results[j]["out"], np.float32).reshape(C, H, W)
        for d, t in enumerate(TAU_BLOCK):
            out[j, t * GS:(t + 1) * GS] = dev[d * GS:(d + 1) * GS]
    return out


# how many 512-px N-tiles are accumulated per psum set before evacuation
CONV_NT = 2


def build_nc():
    from concourse import bacc, mybir, tile
    from contextlib import ExitStack

    dt = mybir.dt
    AF = mybir.ActivationFunctionType
    ALU = mybir.AluOpType

    nc = bacc.Bacc(num_devices=N)

    pred_pad = nc.declare_dram_parameter("pred_pad", [C, PA], dt.bfloat16, isOutput=False)
    patches = nc.declare_dram_parameter("patches", [128, 16 * N * NTAP], dt.bfloat16, isOutput=False)
    styleT = nc.declare_dram_parameter("styleT", [128, 4 * N * 16], dt.float32, isOutput=False)
    sel = nc.declare_dram_parameter("sel", [128, N], dt.float32, isOutput=False)
    dwt = nc.declare_dram_parameter("dwt", [KDW, OSH], dt.bfloat16, isOutput=False)
    dwb = nc.declare_dram_parameter("dwb", [1, OSH], dt.bfloat16, isOutput=False)
    pwkt = nc.declare_dram_parameter("pwkt", [SD, OSH], dt.bfloat16, isOutput=False)
    pwkb = nc.declare_dram_parameter("pwkb", [1, OSH], dt.bfloat16, isOutput=False)
    pwbt = nc.declare_dram_parameter("pwbt", [128, 4 * C], dt.bfloat16, isOutput=False)
    pwbb = nc.declare_dram_parameter("pwbb", [1, C], dt.bfloat16, isOutput=False)
    out_dev = nc.declare_dram_parameter("out", [C, HW], dt.bfloat16, isOutput=True)

    replica = [list(range(N))]

    with tile.TileContext(nc) as tc, ExitStack() as ctx:
        dram = ctx.enter_context(tc.tile_pool(name="dram", bufs=1, space="DRAM"))
        gen_all = dram.tile([N, 10, OSH], dt.bfloat16, tag="gall")
        gen_out = dram.tile([N, 10, OSH], dt.bfloat16, tag="gout")

        const_p = ctx.enter_context(tc.tile_pool(name="const", bufs=1))
        pred_p = ctx.enter_context(tc.tile_pool(name="pred", bufs=1))
        stats_p = ctx.enter_context(tc.tile_pool(name="stats", bufs=2))
        fker_p = ctx.enter_context(tc.tile_pool(name="fker", bufs=1))
        fload_p = ctx.enter_context(tc.tile_pool(name="fload", bufs=2))
        stage_p = ctx.enter_context(tc.tile_pool(name="stage", bufs=2))
        wstream_p = ctx.enter_context(tc.tile_pool(name="wstream", bufs=6))
        gstg_p = ctx.enter_context(tc.tile_pool(name="gstg", bufs=2))

        # ------------------------------------------------ tiny const loads (gpsimd)
        pt = const_p.tile([128, 16, N * NTAP], dt.bfloat16, tag="pt")
        nc.gpsimd.dma_start(out=pt[:], in_=patches.rearrange("p (kt m) -> p kt m", kt=16))

        st = const_p.tile([128, 4, N * 16], dt.float32, tag="st")
        nc.gpsimd.dma_start(out=st[:], in_=styleT.rearrange("p (kt m) -> p kt m", kt=4))

        sel_sb = const_p.tile([128, N], dt.float32, tag="sel")
        nc.gpsimd.dma_start(out=sel_sb[:], in_=sel[:, :])

        pwbt_sb = const_p.tile([128, 4, C], dt.bfloat16, tag="pwbt")
        nc.gpsimd.dma_start(out=pwbt_sb[:], in_=pwbt.rearrange("p (kt m) -> p kt m", kt=4))

        pwbb_sb = const_p.tile([1, C], dt.bfloat16, tag="pwbb")
        nc.gpsimd.dma_start(out=pwbb_sb[:], in_=pwbb[:, :])

        ones = const_p.tile([1, 128], dt.bfloat16, tag="ones")
        nc.vector.memset(ones[:], 1.0)

        # ------------------------------------------------ sd = mean(style, px)
        # (VectorE, tiny; ready long before the pwk generator needs it)
        sdf = const_p.tile([128, 4, N], dt.float32, tag="sdf")
        sdb = const_p.tile([128, 4, N], dt.bfloat16, tag="sdb")
        sdnb = const_p.tile([128, 4], dt.bfloat16, tag="sdnb")  # own-sample column
        tmp8 = stats_p.tile([128, N], dt.float32, tag="tmp8")
        sdn_f = const_p.tile([128, 4], dt.float32, tag="sdnf")
        for kt in range(4):
            nc.vector.tensor_reduce(
                out=sdf[:, kt, :],
                in_=st[:, kt, :].rearrange("p (n x) -> p n x", x=16),
                axis=mybir.AxisListType.X,
                op=ALU.add,
            )
            nc.vector.tensor_scalar(
                out=sdb[:, kt, :], in0=sdf[:, kt, :], scalar1=1.0 / 16.0,
                scalar2=None, op0=ALU.mult,
            )
            # own sample's sd column (via one-hot sel): sdn = sum_n sdf[:,n]*sel[:,n]
            nc.vector.tensor_tensor(
                out=tmp8[:], in0=sdf[:, kt, :], in1=sel_sb[:], op=ALU.mult
            )
            nc.vector.tensor_reduce(
                out=sdn_f[:, kt:kt + 1], in_=tmp8[:], axis=mybir.AxisListType.X, op=ALU.add
            )
        nc.vector.tensor_scalar(
            out=sdnb[:], in0=sdn_f[:], scalar1=1.0 / 16.0, scalar2=None, op0=ALU.mult
        )

        # ------------------------------------------------ generator phase (PE)
        # 0.5 MiB weight chunks round-robined over the 3 DMA-capable engine
        # queues with 8 wstream buffers: each queue keeps ~2.7 chunks in
        # flight so the HBM stream never stalls on matmul consumption.
        dma_engs = [nc.sync, nc.scalar, nc.gpsimd]
        wt_tiles = []
        HOSH = OSH // 2  # 2048 columns per chunk
        with tc.tile_pool(name="psgen", bufs=8, space="PSUM") as psum_g:
            # bias rows preloaded at t=0 (tiny) so the dw accumulation stop
            # never waits on a late queue position
            biasrow = gstg_p.tile([1, OSH], dt.bfloat16, name="biasrow", tag="br", bufs=1)
            nc.sync.dma_start(out=biasrow[0:1, :], in_=dwb[0:1, :])
            # dw generator: 32 weight chunks, 8 psum blocks
            ps_dw = [psum_g.tile([128, 512], dt.float32, name=f"dwg{b}", tag="g")
                     for b in range(8)]
            ci = 0
            for kt in range(16):
                for hf in range(2):
                    wt = wstream_p.tile([128, HOSH], dt.bfloat16, name="wt", tag="w")
                    dma_engs[ci % 3].dma_start(
                        out=wt[:], in_=dwt[kt * 128:(kt + 1) * 128, hf * HOSH:(hf + 1) * HOSH])
                    wt_tiles.append(wt)
                    ci += 1
                    for bb in range(4):
                        b = 4 * hf + bb
                        nc.tensor.matmul(
                            out=ps_dw[b][:N * NTAP, :],
                            lhsT=pt[:, kt, :],
                            rhs=wt[:, bb * 512:(bb + 1) * 512],
                            start=(kt == 0), stop=False,
                        )
            # pwk chunks follow immediately on the same rotation.  The pwk
            # accumulators live in rows 96:104 of the SAME psum banks as the
            # dw blocks (tile_position col 96) so pwk matmuls never wait on
            # bank releases -> no circular buffer/bank stalls.
            pwk_tiles = []
            for kt in range(4):
                for hf in range(2):
                    wt = wstream_p.tile([128, HOSH], dt.bfloat16, name="wt", tag="w")
                    dma_engs[ci % 3].dma_start(
                        out=wt[:], in_=pwkt[kt * 128:(kt + 1) * 128, hf * HOSH:(hf + 1) * HOSH])
                    wt_tiles.append(wt)
                    pwk_tiles.append(wt)
                    ci += 1
                    for bb in range(4):
                        b = 4 * hf + bb
                        nc.tensor.matmul(
                            out=ps_dw[b][96:96 + N, :],
                            lhsT=sdb[:, kt, :],
                            rhs=wt[:, bb * 512:(bb + 1) * 512],
                            start=(kt == 0), stop=False,
                            skip_group_check=True,
                            tile_position=(0, 96),
                        )

            # img deferral: write one corner element of each img half-range
            # from a late weight chunk.  The img DMA then has a WAW dependency
            # on the stub, and the stub a RAW dependency on the chunk -> the
            # 17.3 MiB of img traffic cannot start until the critical weight
            # stream is nearly done.  (A plain ordering stub doesn't work: the
            # Tile scheduler is dependency-driven, not program-ordered.)
            img = [pred_p.tile([128, PA], dt.bfloat16, name=f"img{p}", tag=f"img{p}")
                   for p in range(4)]
            for p in range(4):
                for hh in range(2):
                    nc.gpsimd.tensor_scalar(
                        out=img[p][0:1, hh * PAH:hh * PAH + 1],
                        in0=wt_tiles[28][0:1, 0:1], scalar1=1.0,
                        scalar2=None, op0=ALU.mult,
                    )
            # img loads on sync/gpsimd only: the Scalar queue must stay clear
            # for the generator psum evacuation copies.
            img_engs = [nc.sync, nc.gpsimd]
            for i, (p, hh) in enumerate((p, hh) for p in range(4) for hh in range(2)):
                img_engs[i % 2].dma_start(
                    out=img[p][:, hh * PAH:hh * PAH + PAH],
                    in_=pred_pad[p * 128:(p + 1) * 128, hh * PAH:hh * PAH + PAH])

            # dw bias rows + psum evacuation + store to gen_all
            for b in range(8):
                nc.tensor.matmul(
                    out=ps_dw[b][:N * NTAP, :],
                    lhsT=ones[:1, :N * NTAP],
                    rhs=biasrow[0:1, b * 512:(b + 1) * 512],
                    start=False, stop=True,
                )
                gsb = gstg_p.tile([N * NTAP, 512], dt.bfloat16, name="gsb", tag="gs")
                nc.scalar.copy(out=gsb[:], in_=ps_dw[b][:N * NTAP, :])
                nc.sync.dma_start(
                    out=gen_all[:, 0:NTAP, b * 512:(b + 1) * 512],
                    in_=gsb[:, :],
                )

            # pwk bias + evacuation.  pwkb reuses the biasrow buffer (bufs=1):
            # its DMA has a WAW wait on the dw bias matmuls, which is exactly
            # when it's needed.
            biasrow2 = gstg_p.tile([1, OSH], dt.bfloat16, name="biasrow", tag="br", bufs=1)
            nc.sync.dma_start(out=biasrow2[0:1, :], in_=pwkb[0:1, :])
            g2sb_last = None
            for b in range(8):
                nc.tensor.matmul(
                    out=ps_dw[b][96:96 + N, :],
                    lhsT=ones[:1, :N],
                    rhs=biasrow2[0:1, b * 512:(b + 1) * 512],
                    start=False, stop=True,
                    skip_group_check=True,
                    tile_position=(0, 96),
                )
                g2sb = gstg_p.tile([N, 512], dt.bfloat16, name="g2sb", tag="gs2")
                nc.scalar.copy(out=g2sb[:], in_=ps_dw[b][96:96 + N, :])
                nc.sync.dma_start(
                    out=gen_all[:, NTAP, b * 512:(b + 1) * 512], in_=g2sb[:]
                )
                g2sb_last = g2sb

            # pwb bias chain (device channel order)
            pwb_sb = const_p.tile([128, 4], dt.float32, tag="pwbv")
            for m in range(4):
                pm = psum_g.tile([128, 512], dt.float32, name="pwbps", tag="g")
                for kt in range(4):
                    nc.tensor.matmul(
                        out=pm[:, 0:1],
                        lhsT=pwbt_sb[:, kt, m * 128:(m + 1) * 128],
                        rhs=sdnb[:, kt:kt + 1],
                        start=(kt == 0), stop=False,
                    )
                nc.tensor.matmul(
                    out=pm[:, 0:1],
                    lhsT=pwbb_sb[:1, m * 128:(m + 1) * 128],
                    rhs=ones[:1, 0:1],
                    start=False, stop=True,
                )
                nc.scalar.copy(out=pwb_sb[:, m:m + 1], in_=pm[:, 0:1])

            nc.gpsimd.collective_compute(
                "AllToAll",
                ALU.bypass,
                replica_groups=replica,
                ins=[gen_all[:, :, :].opt()],
                outs=[gen_out[:, :, :].opt()],
            )

        # ------------------------------------------------ instance-norm stats
        # var = E[x^2] - mu^2 (+ ddof correction).  Sum-of-squares via STT with
        # per-chunk accum, plain sum via tensor_reduce, both split at the img
        # half-DMA boundary (pad rows 0-64 | 65-129) so they overlap arrival.
        rstd_sb = const_p.tile([128, 4], dt.float32, tag="rstd")
        muneg_sb = const_p.tile([128, 4], dt.bfloat16, tag="muneg")

        # per-pair accumulators (pairs 2/3 fill theirs mid-conv)
        acc8 = {p: stats_p.tile([128, 8], dt.float32, name=f"acc8_{p}", tag=f"acc8_{p}")
                for p in range(4)}
        ssumc = {p: stats_p.tile([128, 8], dt.float32, name=f"ssum_{p}", tag=f"ssum_{p}")
                 for p in range(4)}
        sqd_v = stats_p.tile([128, 16, 128], dt.bfloat16, name="sqdv", tag="sqdv", bufs=1)
        sqd_s = stats_p.tile([128, 16, 128], dt.bfloat16, name="sqds", tag="sqds", bufs=1)
        # gate: ScalarE stats chunks (pairs 2/3, all WAW on sqd_s) must not be
        # scheduler-hoisted ahead of the generator psum evacuations -> stub a
        # dependency on the last evacuation copy.
        nc.scalar.copy(out=sqd_s[0:1, 0, 0:1], in_=g2sb_last[0:1, 0:1])

        def stats_sq_chunk(p, j, eng, trash):
            """accumulate sum(x^2) of 16-row chunk j into acc8[p][:, j]."""
            view = img[p].rearrange("p (r c) -> p r c", c=PW)
            xs = view[:, 1 + 16 * j:1 + 16 * (j + 1), 1:129]
            if eng is nc.vector:
                eng.scalar_tensor_tensor(
                    out=trash[:], in0=xs, scalar=1.0, in1=xs,
                    op0=ALU.mult, op1=ALU.mult,
                    accum_out=acc8[p][:, j:j + 1],
                )
            else:
                eng.activation(
                    out=trash[:], in_=xs, func=AF.Square,
                    accum_out=acc8[p][:, j:j + 1],
                )

        def stats_sum_quarter(p, q):
            """sum(x) over 32-row quarter q into ssumc[p][:, q] (VectorE)."""
            view = img[p].rearrange("p (r c) -> p r c", c=PW)
            nc.vector.tensor_reduce(
                out=ssumc[p][:, q:q + 1], in_=view[:, 1 + 32 * q:1 + 32 * (q + 1), 1:129],
                axis=mybir.AxisListType.XY, op=ALU.add
            )

        def stats_sum_chunk_scalar(p, j):
            """sum(x) of 16-row chunk j into ssumc[p][:, j] (ScalarE, Copy
            activation with accum; trash via sqd_s so the sqd_s gate applies)."""
            view = img[p].rearrange("p (r c) -> p r c", c=PW)
            xs = view[:, 1 + 16 * j:1 + 16 * (j + 1), 1:129]
            nc.scalar.activation(
                out=sqd_s[:], in_=xs, func=AF.Copy,
                accum_out=ssumc[p][:, j:j + 1],
            )

        def stats_finish(p, nsum):
            """tiny finals: VectorE + one ScalarE sqrt (gated behind the
            generator evacuations so it can't head-block the Scalar queue)."""
            ssq = stats_p.tile([128, 1], dt.float32, name=f"ssq{p}", tag=f"ssq{p}")
            nc.vector.tensor_reduce(
                out=ssq[:], in_=acc8[p][:], axis=mybir.AxisListType.X, op=ALU.add
            )
            ssum = stats_p.tile([128, 1], dt.float32, name=f"ssm{p}", tag=f"ssm{p}")
            nc.vector.tensor_reduce(
                out=ssum[:], in_=ssumc[p][:, 0:nsum], axis=mybir.AxisListType.X, op=ALU.add
            )
            mu = stats_p.tile([128, 1], dt.float32, name=f"mu{p}", tag=f"mu{p}")
            nc.vector.tensor_scalar(
                out=mu[:], in0=ssum[:], scalar1=1.0 / HW, scalar2=None, op0=ALU.mult
            )
            nc.vector.tensor_scalar(
                out=muneg_sb[:, p:p + 1], in0=mu[:], scalar1=-1.0,
                scalar2=None, op0=ALU.mult,
            )
            ex2 = stats_p.tile([128, 1], dt.float32, name=f"ex2_{p}", tag=f"ex2_{p}")
            nc.vector.tensor_scalar(
                out=ex2[:], in0=ssq[:], scalar1=1.0 / HW, scalar2=None, op0=ALU.mult
            )
            mu2 = stats_p.tile([128, 1], dt.float32, name=f"mu2_{p}", tag=f"mu2_{p}")
            nc.vector.tensor_tensor(out=mu2[:], in0=mu[:], in1=mu[:], op=ALU.mult)
            varp = stats_p.tile([128, 1], dt.float32, name=f"varp{p}", tag=f"varp{p}")
            nc.vector.tensor_tensor(out=varp[:], in0=ex2[:], in1=mu2[:], op=ALU.subtract)
            vtmp = stats_p.tile([128, 1], dt.float32, name=f"vt{p}", tag=f"vt{p}")
            nc.vector.tensor_scalar(
                out=vtmp[:], in0=varp[:], scalar1=VAR_CORR, scalar2=EPS,
                op0=ALU.mult, op1=ALU.add,
            )
            stdt = stats_p.tile([128, 1], dt.float32, name=f"st{p}", tag=f"st{p}")
            nc.scalar.copy(out=stdt[0:1, 0:1], in_=g2sb_last[0:1, 0:1])  # gate
            nc.scalar.sqrt(stdt[:], vtmp[:])
            nc.vector.reciprocal(out=rstd_sb[:, p:p + 1], in_=stdt[:])

        # pairs 0/1: VectorE takes x^2, ScalarE takes sum(x) (gated behind the
        # generator evacuations via sqd_s), overlapping the generator phase
        for p in (0, 1):
            for j in range(8):
                stats_sq_chunk(p, j, nc.vector, sqd_v)
                stats_sum_chunk_scalar(p, j)
            stats_finish(p, 8)

        # ------------------------------------------------ per-half: F build + conv
        with tc.tile_pool(name="pssml", bufs=2, space="PSUM") as psum_s, \
             tc.tile_pool(name="psconv", bufs=6, space="PSUM") as psum_c:

            def fbuild(h):
                """Gather generated kernels + build fused, rstd-scaled F."""
                pA, pB = 2 * h, 2 * h + 1
                # one strided DMA per group quadrant -> [part=c1', (tap|pwk), ci/co]
                wA = fload_p.tile([128, 10, GS], dt.bfloat16, name="wA", tag="wld")
                wB = fload_p.tile([128, 10, GS], dt.bfloat16, name="wB", tag="wld")
                for w128, g0 in ((wA, 4 * h), (wB, 4 * h + 2)):
                    nc.sync.dma_start(
                        out=w128[0:64, :, :],
                        in_=gen_out[g0, :, :].rearrange("t (c s) -> c t s", c=64),
                    )
                    nc.sync.dma_start(
                        out=w128[64:128, :, :],
                        in_=gen_out[g0 + 1, :, :].rearrange("t (c s) -> c t s", c=64),
                    )
                # fused, scaled kernels F~ : [part(ci), tap, geo, co2] bf16
                fsb = fker_p.tile([128, NTAP, 2, GS], dt.bfloat16,
                                  name=f"fsb{h}", tag=f"fsb{h}")
                for t in range(NTAP):
                    psA = psum_s.tile([128, 512], dt.float32, name="fbA", tag="s")
                    psB = psum_s.tile([128, 512], dt.float32, name="fbB", tag="s")
                    # F^T = W^T @ P^T  (fp32), per group quadrant
                    nc.tensor.matmul(out=psA[0:64, :GS], lhsT=wA[0:64, t, :],
                                     rhs=wA[0:64, NTAP, :], start=True, stop=True)
                    nc.tensor.matmul(out=psA[64:128, :GS], lhsT=wA[64:128, t, :],
                                     rhs=wA[64:128, NTAP, :], start=True, stop=True)
                    nc.tensor.matmul(out=psB[0:64, :GS], lhsT=wB[0:64, t, :],
                                     rhs=wB[0:64, NTAP, :], start=True, stop=True)
                    nc.tensor.matmul(out=psB[64:128, :GS], lhsT=wB[64:128, t, :],
                                     rhs=wB[64:128, NTAP, :], start=True, stop=True)
                    # scale by rstd (per input channel) + cast bf16
                    nc.vector.tensor_scalar(out=fsb[:, t, 0, :], in0=psA[:, :GS],
                                            scalar1=rstd_sb[:, pA:pA + 1],
                                            scalar2=None, op0=ALU.mult)
                    nc.vector.tensor_scalar(out=fsb[:, t, 1, :], in0=psB[:, :GS],
                                            scalar1=rstd_sb[:, pB:pB + 1],
                                            scalar2=None, op0=ALU.mult)
                return fsb

            def bchain(h, fsb):
                """B bias accumulation (psum holds -B); pair B quadrant-swapped."""
                pA, pB = 2 * h, 2 * h + 1
                bpsA = psum_s.tile([128, 512], dt.float32, name="bpsA", tag="s")
                bpsB = psum_s.tile([128, 512], dt.float32, name="bpsB", tag="s")
                for t in range(NTAP):
                    st_ = (t == 0)
                    sp_ = (t == NTAP - 1)
                    nc.tensor.matmul(out=bpsA[0:64, 0:1], lhsT=fsb[0:64, t, 0, :],
                                     rhs=muneg_sb[0:64, pA:pA + 1], start=st_, stop=sp_,
                                     skip_group_check=True)
                    nc.tensor.matmul(out=bpsA[64:128, 0:1], lhsT=fsb[64:128, t, 0, :],
                                     rhs=muneg_sb[64:128, pA:pA + 1], start=st_, stop=sp_,
                                     skip_group_check=True)
                    nc.tensor.matmul(out=bpsB[64:128, 0:1], lhsT=fsb[0:64, t, 1, :],
                                     rhs=muneg_sb[0:64, pB:pB + 1], start=st_, stop=sp_,
                                     skip_group_check=True)
                    nc.tensor.matmul(out=bpsB[0:64, 0:1], lhsT=fsb[64:128, t, 1, :],
                                     rhs=muneg_sb[64:128, pB:pB + 1], start=st_, stop=sp_,
                                     skip_group_check=True)
                bias_A = const_p.tile([128, 1], dt.float32, name=f"biasA{h}", tag=f"bA{h}")
                bias_B = const_p.tile([128, 1], dt.float32, name=f"biasB{h}", tag=f"bB{h}")
                nc.vector.tensor_tensor(out=bias_A[:], in0=bpsA[:, 0:1],
                                        in1=pwb_sb[:, pA:pA + 1], op=ALU.add)
                nc.vector.tensor_tensor(out=bias_B[:], in0=bpsB[:, 0:1],
                                        in1=pwb_sb[:, pB:pB + 1], op=ALU.add)
                return bias_A, bias_B

            h1_state = {}

            def conv_half(h, fsb, bias_A, bias_B):
                pA, pB = 2 * h, 2 * h + 1
                imA = img[pA].rearrange("p (r c) -> p r c", c=PW)
                imB = img[pB].rearrange("p (r c) -> p r c", c=PW)
                NBLK = HW // (512 * CONV_NT)
                for blk in range(NBLK):
                    y0 = blk * 4 * CONV_NT
                    psA_ = [psum_c.tile([128, 512], dt.float32, name="cvA", tag="c")
                            for _ in range(CONV_NT)]
                    psB_ = [psum_c.tile([128, 512], dt.float32, name="cvB", tag="c")
                            for _ in range(CONV_NT)]
                    for t in range(NTAP):
                        ky, kx = t // 3, t % 3
                        st_ = (t == 0)
                        sp_ = (t == NTAP - 1)
                        for nt in range(CONV_NT):
                            y = y0 + 4 * nt
                            rA = imA[:, y + ky:y + ky + 4, kx:kx + 128]
                            rB = imB[:, y + ky:y + ky + 4, kx:kx + 128]
                            nc.tensor.matmul(out=psA_[nt][0:64, :], lhsT=fsb[0:64, t, 0, :],
                                             rhs=rA[0:64], start=st_, stop=sp_,
                                             skip_group_check=True)
                            nc.tensor.matmul(out=psA_[nt][64:128, :], lhsT=fsb[64:128, t, 0, :],
                                             rhs=rA[64:128], start=st_, stop=sp_,
                                             skip_group_check=True)
                            nc.tensor.matmul(out=psB_[nt][64:128, :], lhsT=fsb[0:64, t, 1, :],
                                             rhs=rB[0:64], start=st_, stop=sp_,
                                             skip_group_check=True)
                            nc.tensor.matmul(out=psB_[nt][0:64, :], lhsT=fsb[64:128, t, 1, :],
                                             rhs=rB[64:128], start=st_, stop=sp_,
                                             skip_group_check=True)
                    # evacuate + bias: ScalarE takes pair A, VectorE pair B;
                    # staged bf16 to halve the write-back DMA.
                    stA = stage_p.tile([128, 512 * CONV_NT], dt.bfloat16, name="stA", tag="stg")
                    stB = stage_p.tile([128, 512 * CONV_NT], dt.bfloat16, name="stB", tag="stg")
                    for nt in range(CONV_NT):
                        nc.scalar.activation(
                            out=stA[:, nt * 512:(nt + 1) * 512], in_=psA_[nt][:],
                            func=AF.Identity, bias=bias_A[:, 0:1], scale=1.0,
                        )
                        nc.vector.tensor_scalar(
                            out=stB[:, nt * 512:(nt + 1) * 512], in0=psB_[nt][:],
                            scalar1=bias_B[:, 0:1], scalar2=None, op0=ALU.add,
                        )
                    px0 = y0 * 128
                    nc.sync.dma_start(
                        out=out_dev[pA * 128:(pA + 1) * 128, px0:px0 + 512 * CONV_NT],
                        in_=stA[:],
                    )
                    nc.sync.dma_start(
                        out=out_dev[pB * 128:(pB + 1) * 128, px0:px0 + 512 * CONV_NT],
                        in_=stB[:],
                    )

                    if h == 0:
                        # pairs 2/3 stats interleaved with conv h=0: ScalarE
                        # takes the x^2 chunks, VectorE the sum quarters,
                        # slotted between eviction batches so neither queue
                        # head-blocks; then F-build h=1 in the tail blocks so
                        # the h0->h1 transition has no PE bubble.
                        if 2 <= blk < 10:
                            p23 = 2 + (blk - 2) // 4
                            q23 = (blk - 2) % 4
                            stats_sq_chunk(p23, 2 * q23, nc.scalar, sqd_s)
                            stats_sq_chunk(p23, 2 * q23 + 1, nc.scalar, sqd_s)
                            stats_sum_quarter(p23, q23)
                        elif blk == 10:
                            stats_finish(2, 4)
                        elif blk == 11:
                            stats_finish(3, 4)
                        elif blk == 13:
                            h1_state["fsb"] = fbuild(1)
                        elif blk == 15:
                            h1_state["bias"] = bchain(1, h1_state["fsb"])

            fsb0 = fbuild(0)
            # gate: VectorE sum-quarters for pairs 2/3 wait for F-build h0's
            # scales (same WAW-stub trick; keeps F h0 off the back of a
            # hoisted 12us stats burst).
            for p23 in (2, 3):
                nc.vector.tensor_scalar(
                    out=ssumc[p23][0:1, :], in0=fsb0[0:1, 0, 0, 0:8],
                    scalar1=0.0, scalar2=None, op0=ALU.mult,
                )
            bias_A0, bias_B0 = bchain(0, fsb0)
            conv_half(0, fsb0, bias_A0, bias_B0)
            conv_half(1, h1_state["fsb"], *h1_state["bias"])

    nc.compile()
    return nc


_NC_CACHE = {}


def kernel(**inputs) -> np.ndarray:
    from concourse.bass_utils import run_bass_kernel_spmd

    in_maps = _host_prep(**inputs)
    if "nc" not in _NC_CACHE:
        _NC_CACHE["nc"] = build_nc()
    nc = _NC_CACHE["nc"]
    res = run_bass_kernel_spmd(nc, in_maps, core_ids=list(range(N)))
    return _unshard(res.results)


if __name__ == "__main__":
    import jax

    import reference

    with jax.default_device(jax.devices("cpu")[0]):
        inputs = {k: np.asarray(v) for k, v in reference.setup_inputs().items()}
        expected = np.asarray(reference.reference(**inputs))
    actual = kernel(**inputs)
    err = np.sqrt(((actual - expected) ** 2).mean()) / np.sqrt((expected ** 2).mean())
    print("Relative error:", err)


# revision 52
# speedup vs baseline: 1.0973x; 1.0973x over previous
"""AdaConv (nn_AdaConv_46445776339355) — 8-core TRN2 Bass kernel.

Strategy
--------
Data-parallel over batch N=8: core n owns sample n end-to-end for the heavy
instance-norm + grouped-conv work.  The kernel *generator* (dw_w is 256 MiB)
is tensor-parallel: core j holds the output-channel shard j of dw_w / pwk_w,
computes the generated kernels for ALL samples on its shard, and an AllToAll
routes each sample's kernels to its owning core.

Algebraic fusions (all computed on device):
  * pointwise o depthwise = one fused per-group kernel  F_t = P @ W_t
  * instance norm folded into the fused kernels:
        y = sum_t F_t @ ((x-mu)/sigma)_pad = sum_t (F_t/sigma_ci) @ x_pad - B
    with B = sum_t (F_t/sigma) @ mu  (position independent, reflect-pad safe)
  * biases (dw_b, pwk_b, pwb_b) folded in via K=1 matmul rows.

The grouped conv (8 groups of 64->64 ch, 3x3) runs as 4 concurrent 64x64
matmuls in the 4 PE-array quadrants (tile_position packing) => full 128x128
PE utilization, bf16, 9 shifted-AP taps accumulating in PSUM.

v2 flow restructure (from trace analysis of the 455us baseline):
  * generator weight chunks stream first on the sync/scalar DMA queues with
    nothing queued ahead of them (the 21 MiB stream is the serial critical
    path: ~60us at full BW); pwk chunk DMAs are hoisted before the PSUM
    evacuation copies so the stream never stalls behind compute waits.
  * img (predicted) loads are deferred behind the weight stream via a
    dependency stub on a late weight chunk, in 2 half-tile DMAs each so the
    stats reductions overlap arrival.
  * instance-norm stats for pairs 0/1 run on VectorE during the generator
    phase; pairs 2/3 run on GpSimdE during conv h=0 (VectorE evicts PSUM).
  * PSUM eviction split: ScalarE takes pair A, VectorE pair B; output staged
    bf16 (halves write-back DMA; rel-err budget is loose).
  * F-build gathers collapsed to 4 strided DMAs per half.
"""

import sys
import numpy as np

sys.path.insert(0, "/opt/trn_rl_repo")

import ml_dtypes

BF16 = ml_dtypes.bfloat16

# ---------------- problem constants (hardcoded per the harness contract) ----
N = 8            # batch == number of cores
C = 512          # channels
H = W = 128
HW = H * W       # 16384
PW = W + 2       # 130 padded
PA = PW * PW     # 16900
PAH = 65 * PW    # 8450: half-tile split (pad rows 0-64 | 65-129)
SD = 512         # style dim
NG = 8           # groups
GS = 64          # group size (channels per group)
KDW = SD * 4     # 2048 contraction dim of the dw generator
OSH = 4096       # dw/pwk output-channel shard per core (32768/8); == one group block
NTAP = 9
EPS = 1e-5
VAR_CORR = float(HW) / float(HW - 1)  # ddof=1 correction

# device output channel order: per pair of groups (2h, 2h+1) natural, the odd
# pairs (pB) have their two 64-blocks swapped (quadrant output packing).
TAU_BLOCK = [0, 1, 3, 2, 4, 5, 7, 6]  # true 64-block of device 64-block d


def _host_prep(style_encoding, predicted, dw_w, dw_b, pwk_w, pwk_b, pwb_w, pwb_b):
    """Pure data-movement / dtype-cast host prep. Returns per-core input maps."""
    f32 = np.float32
    se = np.asarray(style_encoding, f32)
    pred = np.asarray(predicted, f32)

    # --- patches for the dw generator conv: reflect pad 1, 2x2 windows s=2 ---
    sep = np.pad(se, ((0, 0), (0, 0), (1, 1), (1, 1)), mode="reflect")  # (8,512,6,6)
    blocks = sep.reshape(N, SD, 3, 2, 3, 2)  # [n,c,oy,ky,ox,kx]
    patches = np.ascontiguousarray(
        blocks.transpose(1, 3, 5, 0, 2, 4).reshape(KDW, N * NTAP)
    ).astype(BF16)  # [(c,ky,kx), (n,oy,ox)] = [2048, 72]
    # pre-tiled for a contiguous SBUF DMA: [128, kt, m]
    patches_t = np.ascontiguousarray(
        patches.reshape(16, 128, N * NTAP).transpose(1, 0, 2).reshape(128, 16 * N * NTAP)
    )

    # --- dw generator weights, transposed + sharded on output channels ---
    dwt_full = np.ascontiguousarray(dw_w.reshape(C * GS, KDW).T).astype(BF16)  # [2048, 32768]
    dwb_full = np.asarray(dw_b, f32).reshape(1, C * GS).astype(BF16)

    # --- pwk: permute columns to (g, cm, co2) so the gathered row IS P^T ---
    pwk_t = np.asarray(pwk_w, f32).reshape(NG, GS, GS, SD)  # [g, co2, cm, sd]
    pwkt_full = np.ascontiguousarray(
        pwk_t.transpose(3, 0, 2, 1).reshape(SD, C * GS)
    ).astype(BF16)  # [sd, (g, cm, co2)]
    pwkb_full = (
        np.asarray(pwk_b, f32).reshape(NG, GS, GS).transpose(0, 2, 1).reshape(1, C * GS)
    ).astype(BF16)

    # --- pwb: transposed, columns in DEVICE channel order tau ---
    tau_rows = np.concatenate([np.arange(GS) + t * GS for t in TAU_BLOCK])  # [512]
    pwbt = np.ascontiguousarray(np.asarray(pwb_w, f32)[tau_rows, :].T).astype(BF16)  # [sd, out_dev]
    pwbt_t = np.ascontiguousarray(
        pwbt.reshape(4, 128, C).transpose(1, 0, 2).reshape(128, 4 * C))
    pwbb = np.asarray(pwb_b, f32)[tau_rows].reshape(1, C).astype(BF16)

    # --- styleT for sd computation on device: pre-tiled [128, kt, (n, px)] f32 ---
    styleT = np.ascontiguousarray(se.transpose(1, 0, 2, 3).reshape(SD, N * 16)).astype(f32)
    styleT_t = np.ascontiguousarray(
        styleT.reshape(4, 128, N * 16).transpose(1, 0, 2).reshape(128, 4 * N * 16))

    in_maps = []
    for j in range(N):
        pp = np.pad(pred[j], ((0, 0), (1, 1), (1, 1)), mode="reflect").reshape(C, PA)
        sel = np.zeros((128, N), f32)
        sel[:, j] = 1.0
        in_maps.append(
            dict(
                pred_pad=np.ascontiguousarray(pp).astype(BF16),
                patches=patches_t,
                styleT=styleT_t,
                sel=sel,
                dwt=np.ascontiguousarray(dwt_full[:, j * OSH:(j + 1) * OSH]),
                dwb=np.ascontiguousarray(dwb_full[:, j * OSH:(j + 1) * OSH]),
                pwkt=np.ascontiguousarray(pwkt_full[:, j * OSH:(j + 1) * OSH]),
                pwkb=np.ascontiguousarray(pwkb_full[:, j * OSH:(j + 1) * OSH]),
                pwbt=pwbt_t,
                pwbb=pwbb,
            )
        )
    return in_maps


def _unshard(results):
    """results[j]['out'] is [512, 16384] bf16 in device channel order."""
    out = np.empty((N, C, H, W), np.float32)
    for j in range(N):
        dev = np.asarray(results[j]["out"], np.float32).reshape(C, H, W)
        for d, t in enumerate(TAU_BLOCK):
            out[j, t * GS:(t + 1) * GS] = dev[d * GS:(d + 1) * GS]
    return out


# how many 512-px N-tiles are accumulated per psum set before evacuation
CONV_NT = 2


def build_nc():
    from concourse import bacc, mybir, tile
    from contextlib import ExitStack

    dt = mybir.dt
    AF = mybir.ActivationFunctionType
    ALU = mybir.AluOpType

    nc = bacc.Bacc(num_devices=N)

    pred_pad = nc.declare_dram_parameter("pred_pad", [C, PA], dt.bfloat16, isOutput=False)
    patches = nc.declare_dram_parameter("patches", [128, 16 * N * NTAP], dt.bfloat16, isOutput=False)
    styleT = nc.declare_dram_parameter("styleT", [128, 4 * N * 16], dt.float32, isOutput=False)
    sel = nc.declare_dram_parameter("sel", [128, N], dt.float32, isOutput=False)
    dwt = nc.declare_dram_parameter("dwt", [KDW, OSH], dt.bfloat16, isOutput=False)
    dwb = nc.declare_dram_parameter("dwb", [1, OSH], dt.bfloat16, isOutput=False)
    pwkt = nc.declare_dram_parameter("pwkt", [SD, OSH], dt.bfloat16, isOutput=False)
    pwkb = nc.declare_dram_parameter("pwkb", [1, OSH], dt.bfloat16, isOutput=False)
    pwbt = nc.declare_dram_parameter("pwbt", [128, 4 * C], dt.bfloat16, isOutput=False)
    pwbb = nc.declare_dram_parameter("pwbb", [1, C], dt.bfloat16, isOutput=False)
    out_dev = nc.declare_dram_parameter("out", [C, HW], dt.bfloat16, isOutput=True)

    replica = [list(range(N))]

    with tile.TileContext(nc) as tc, ExitStack() as ctx:
        dram = ctx.enter_context(tc.tile_pool(name="dram", bufs=1, space="DRAM"))
        gen_all = dram.tile([N, 10, OSH], dt.bfloat16, tag="gall")
        gen_out = dram.tile([N, 10, OSH], dt.bfloat16, tag="gout")

        const_p = ctx.enter_context(tc.tile_pool(name="const", bufs=1))
        pred_p = ctx.enter_context(tc.tile_pool(name="pred", bufs=1))
        stats_p = ctx.enter_context(tc.tile_pool(name="stats", bufs=2))
        fker_p = ctx.enter_context(tc.tile_pool(name="fker", bufs=1))
        fload_p = ctx.enter_context(tc.tile_pool(name="fload", bufs=2))
        stage_p = ctx.enter_context(tc.tile_pool(name="stage", bufs=2))
        wstream_p = ctx.enter_context(tc.tile_pool(name="wstream", bufs=6))
        gstg_p = ctx.enter_context(tc.tile_pool(name="gstg", bufs=2))

        # ------------------------------------------------ tiny const loads (gpsimd)
        pt = const_p.tile([128, 16, N * NTAP], dt.bfloat16, tag="pt")
        nc.gpsimd.dma_start(out=pt[:], in_=patches.rearrange("p (kt m) -> p kt m", kt=16))

        st = const_p.tile([128, 4, N * 16], dt.float32, tag="st")
        nc.gpsimd.dma_start(out=st[:], in_=styleT.rearrange("p (kt m) -> p kt m", kt=4))

        sel_sb = const_p.tile([128, N], dt.float32, tag="sel")
        nc.gpsimd.dma_start(out=sel_sb[:], in_=sel[:, :])

        pwbt_sb = const_p.tile([128, 4, C], dt.bfloat16, tag="pwbt")
        nc.gpsimd.dma_start(out=pwbt_sb[:], in_=pwbt.rearrange("p (kt m) -> p kt m", kt=4))

        pwbb_sb = const_p.tile([1, C], dt.bfloat16, tag="pwbb")
        nc.gpsimd.dma_start(out=pwbb_sb[:], in_=pwbb[:, :])

        ones = const_p.tile([1, 128], dt.bfloat16, tag="ones")
        nc.vector.memset(ones[:], 1.0)

        # ------------------------------------------------ sd = mean(style, px)
        # (VectorE, tiny; ready long before the pwk generator needs it)
        sdf = const_p.tile([128, 4, N], dt.float32, tag="sdf")
        sdb = const_p.tile([128, 4, N], dt.bfloat16, tag="sdb")
        sdnb = const_p.tile([128, 4], dt.bfloat16, tag="sdnb")  # own-sample column
        tmp8 = stats_p.tile([128, N], dt.float32, tag="tmp8")
        sdn_f = const_p.tile([128, 4], dt.float32, tag="sdnf")
        for kt in range(4):
            nc.vector.tensor_reduce(
                out=sdf[:, kt, :],
                in_=st[:, kt, :].rearrange("p (n x) -> p n x", x=16),
                axis=mybir.AxisListType.X,
                op=ALU.add,
            )
            nc.vector.tensor_scalar(
                out=sdb[:, kt, :], in0=sdf[:, kt, :], scalar1=1.0 / 16.0,
                scalar2=None, op0=ALU.mult,
            )
            # own sample's sd column (via one-hot sel): sdn = sum_n sdf[:,n]*sel[:,n]
            nc.vector.tensor_tensor(
                out=tmp8[:], in0=sdf[:, kt, :], in1=sel_sb[:], op=ALU.mult
            )
            nc.vector.tensor_reduce(
                out=sdn_f[:, kt:kt + 1], in_=tmp8[:], axis=mybir.AxisListType.X, op=ALU.add
            )
        nc.vector.tensor_scalar(
            out=sdnb[:], in0=sdn_f[:], scalar1=1.0 / 16.0, scalar2=None, op0=ALU.mult
        )

        # ------------------------------------------------ generator phase (PE)
        # 0.5 MiB weight chunks round-robined over the 3 DMA-capable engine
        # queues with 8 wstream buffers: each queue keeps ~2.7 chunks in
        # flight so the HBM stream never stalls on matmul consumption.
        dma_engs = [nc.sync, nc.scalar, nc.gpsimd]
        wt_tiles = []
        HOSH = OSH // 2  # 2048 columns per chunk
        with tc.tile_pool(name="psgen", bufs=8, space="PSUM") as psum_g:
            # bias rows preloaded at t=0 (tiny) so the dw accumulation stop
            # never waits on a late queue position
            biasrow = gstg_p.tile([1, OSH], dt.bfloat16, name="biasrow", tag="br", bufs=1)
            nc.sync.dma_start(out=biasrow[0:1, :], in_=dwb[0:1, :])
            biasrow2 = gstg_p.tile([1, OSH], dt.bfloat16, name="biasrow2", tag="br2", bufs=1)
            nc.scalar.dma_start(out=biasrow2[0:1, :], in_=pwkb[0:1, :])
            # dw generator: 32 weight chunks, 8 psum blocks
            ps_dw = [psum_g.tile([128, 512], dt.float32, name=f"dwg{b}", tag="g")
                     for b in range(8)]
            ci = 0
            for kt in range(16):
                for hf in range(2):
                    wt = wstream_p.tile([128, HOSH], dt.bfloat16, name="wt", tag="w")
                    dma_engs[ci % 3].dma_start(
                        out=wt[:], in_=dwt[kt * 128:(kt + 1) * 128, hf * HOSH:(hf + 1) * HOSH])
                    wt_tiles.append(wt)
                    ci += 1
                    for bb in range(4):
                        b = 4 * hf + bb
                        nc.tensor.matmul(
                            out=ps_dw[b][:N * NTAP, :],
                            lhsT=pt[:, kt, :],
                            rhs=wt[:, bb * 512:(bb + 1) * 512],
                            start=(kt == 0), stop=False,
                        )
            # pwk chunks follow immediately on the same rotation.  The pwk
            # accumulators live in rows 96:104 of the SAME psum banks as the
            # dw blocks (tile_position col 96) so pwk matmuls never wait on
            # bank releases -> no circular buffer/bank stalls.
            pwk_tiles = []
            for kt in range(4):
                for hf in range(2):
                    wt = wstream_p.tile([128, HOSH], dt.bfloat16, name="wt", tag="w")
                    dma_engs[ci % 3].dma_start(
                        out=wt[:], in_=pwkt[kt * 128:(kt + 1) * 128, hf * HOSH:(hf + 1) * HOSH])
                    wt_tiles.append(wt)
                    pwk_tiles.append(wt)
                    ci += 1
                    for bb in range(4):
                        b = 4 * hf + bb
                        nc.tensor.matmul(
                            out=ps_dw[b][96:96 + N, :],
                            lhsT=sdb[:, kt, :],
                            rhs=wt[:, bb * 512:(bb + 1) * 512],
                            start=(kt == 0), stop=False,
                            skip_group_check=True,
                            tile_position=(0, 96),
                        )

            # img deferral: write one corner element of each img half-range
            # from a late weight chunk.  The img DMA then has a WAW dependency
            # on the stub, and the stub a RAW dependency on the chunk -> the
            # 17.3 MiB of img traffic cannot start until the critical weight
            # stream is nearly done.  (A plain ordering stub doesn't work: the
            # Tile scheduler is dependency-driven, not program-ordered.)
            img = [pred_p.tile([128, PA], dt.bfloat16, name=f"img{p}", tag=f"img{p}")
                   for p in range(4)]
            for p in range(4):
                for hh in range(2):
                    nc.gpsimd.tensor_scalar(
                        out=img[p][0:1, hh * PAH:hh * PAH + 1],
                        in0=wt_tiles[28][0:1, 0:1], scalar1=1.0,
                        scalar2=None, op0=ALU.mult,
                    )
            # img loads on sync/gpsimd only: the Scalar queue must stay clear
            # for the generator psum evacuation copies.
            img_engs = [nc.sync, nc.gpsimd]
            for i, (p, hh) in enumerate((p, hh) for p in range(4) for hh in range(2)):
                img_engs[i % 2].dma_start(
                    out=img[p][:, hh * PAH:hh * PAH + PAH],
                    in_=pred_pad[p * 128:(p + 1) * 128, hh * PAH:hh * PAH + PAH])

            # dw bias rows + psum evacuation + store to gen_all
            for b in range(8):
                nc.tensor.matmul(
                    out=ps_dw[b][:N * NTAP, :],
                    lhsT=ones[:1, :N * NTAP],
                    rhs=biasrow[0:1, b * 512:(b + 1) * 512],
                    start=False, stop=True,
                )
                gsb = gstg_p.tile([N * NTAP, 512], dt.bfloat16, name="gsb", tag="gs")
                nc.vector.tensor_copy(out=gsb[:], in_=ps_dw[b][:N * NTAP, :])
                nc.sync.dma_start(
                    out=gen_all[:, 0:NTAP, b * 512:(b + 1) * 512],
                    in_=gsb[:, :],
                )

            # pwk bias + evacuation
            g2sb_last = None
            for b in range(8):
                nc.tensor.matmul(
                    out=ps_dw[b][96:96 + N, :],
                    lhsT=ones[:1, :N],
                    rhs=biasrow2[0:1, b * 512:(b + 1) * 512],
                    start=False, stop=True,
                    skip_group_check=True,
                    tile_position=(0, 96),
                )
                g2sb = gstg_p.tile([N, 512], dt.bfloat16, name="g2sb", tag="gs2")
                nc.vector.tensor_copy(out=g2sb[:], in_=ps_dw[b][96:96 + N, :])
                nc.sync.dma_start(
                    out=gen_all[:, NTAP, b * 512:(b + 1) * 512], in_=g2sb[:]
                )
                g2sb_last = g2sb

            # pwb bias chain (device channel order)
            pwb_sb = const_p.tile([128, 4], dt.float32, tag="pwbv")
            for m in range(4):
                pm = psum_g.tile([128, 512], dt.float32, name="pwbps", tag="g")
                for kt in range(4):
                    nc.tensor.matmul(
                        out=pm[:, 0:1],
                        lhsT=pwbt_sb[:, kt, m * 128:(m + 1) * 128],
                        rhs=sdnb[:, kt:kt + 1],
                        start=(kt == 0), stop=False,
                    )
                nc.tensor.matmul(
                    out=pm[:, 0:1],
                    lhsT=pwbb_sb[:1, m * 128:(m + 1) * 128],
                    rhs=ones[:1, 0:1],
                    start=False, stop=True,
                )
                nc.vector.tensor_copy(out=pwb_sb[:, m:m + 1], in_=pm[:, 0:1])

            nc.gpsimd.collective_compute(
                "AllToAll",
                ALU.bypass,
                replica_groups=replica,
                ins=[gen_all[:, :, :].opt()],
                outs=[gen_out[:, :, :].opt()],
            )

        # ------------------------------------------------ instance-norm stats
        # var = E[x^2] - mu^2 (+ ddof correction).  Sum-of-squares via STT with
        # per-chunk accum, plain sum via tensor_reduce, both split at the img
        # half-DMA boundary (pad rows 0-64 | 65-129) so they overlap arrival.
        rstd_sb = const_p.tile([128, 4], dt.float32, tag="rstd")
        muneg_sb = const_p.tile([128, 4], dt.bfloat16, tag="muneg")

        # per-pair accumulators (pairs 2/3 fill theirs mid-conv)
        acc8 = {p: stats_p.tile([128, 8], dt.float32, name=f"acc8_{p}", tag=f"acc8_{p}")
                for p in range(4)}
        ssumc = {p: stats_p.tile([128, 8], dt.float32, name=f"ssum_{p}", tag=f"ssum_{p}")
                 for p in range(4)}
        sqd_v = stats_p.tile([128, 16, 128], dt.bfloat16, name="sqdv", tag="sqdv", bufs=1)
        sqd_s = stats_p.tile([128, 16, 128], dt.bfloat16, name="sqds", tag="sqds", bufs=1)

        def stats_sq_chunk(p, j, eng, trash):
            """accumulate sum(x^2) of 16-row chunk j into acc8[p][:, j]."""
            view = img[p].rearrange("p (r c) -> p r c", c=PW)
            xs = view[:, 1 + 16 * j:1 + 16 * (j + 1), 1:129]
            if eng is nc.vector:
                eng.scalar_tensor_tensor(
                    out=trash[:], in0=xs, scalar=1.0, in1=xs,
                    op0=ALU.mult, op1=ALU.mult,
                    accum_out=acc8[p][:, j:j + 1],
                )
            else:
                eng.activation(
                    out=trash[:], in_=xs, func=AF.Square,
                    accum_out=acc8[p][:, j:j + 1],
                )

        def stats_sum_chunk_scalar(p, j):
            """sum(x) of 16-row chunk j into ssumc[p][:, j] (ScalarE, Copy
            activation with accum; trash via sqd_s so the sqd_s gate applies)."""
            view = img[p].rearrange("p (r c) -> p r c", c=PW)
            xs = view[:, 1 + 16 * j:1 + 16 * (j + 1), 1:129]
            nc.scalar.activation(
                out=sqd_s[:], in_=xs, func=AF.Copy,
                accum_out=ssumc[p][:, j:j + 1],
            )

        def stats_finish(p, nsum):
            """tiny finals: VectorE + one ScalarE sqrt (gated behind the
            generator evacuations so it can't head-block the Scalar queue)."""
            ssq = stats_p.tile([128, 1], dt.float32, name=f"ssq{p}", tag=f"ssq{p}")
            nc.vector.tensor_reduce(
                out=ssq[:], in_=acc8[p][:], axis=mybir.AxisListType.X, op=ALU.add
            )
            ssum = stats_p.tile([128, 1], dt.float32, name=f"ssm{p}", tag=f"ssm{p}")
            nc.vector.tensor_reduce(
                out=ssum[:], in_=ssumc[p][:, 0:nsum], axis=mybir.AxisListType.X, op=ALU.add
            )
            mu = stats_p.tile([128, 1], dt.float32, name=f"mu{p}", tag=f"mu{p}")
            nc.vector.tensor_scalar(
                out=mu[:], in0=ssum[:], scalar1=1.0 / HW, scalar2=None, op0=ALU.mult
            )
            nc.vector.tensor_scalar(
                out=muneg_sb[:, p:p + 1], in0=mu[:], scalar1=-1.0,
                scalar2=None, op0=ALU.mult,
            )
            ex2 = stats_p.tile([128, 1], dt.float32, name=f"ex2_{p}", tag=f"ex2_{p}")
            nc.vector.tensor_scalar(
                out=ex2[:], in0=ssq[:], scalar1=1.0 / HW, scalar2=None, op0=ALU.mult
            )
            mu2 = stats_p.tile([128, 1], dt.float32, name=f"mu2_{p}", tag=f"mu2_{p}")
            nc.vector.tensor_tensor(out=mu2[:], in0=mu[:], in1=mu[:], op=ALU.mult)
            varp = stats_p.tile([128, 1], dt.float32, name=f"varp{p}", tag=f"varp{p}")
            nc.vector.tensor_tensor(out=varp[:], in0=ex2[:], in1=mu2[:], op=ALU.subtract)
            vtmp = stats_p.tile([128, 1], dt.float32, name=f"vt{p}", tag=f"vt{p}")
            nc.vector.tensor_scalar(
                out=vtmp[:], in0=varp[:], scalar1=VAR_CORR, scalar2=EPS,
                op0=ALU.mult, op1=ALU.add,
            )
            stdt = stats_p.tile([128, 1], dt.float32, name=f"st{p}", tag=f"st{p}")
            nc.scalar.sqrt(stdt[:], vtmp[:])
            nc.vector.reciprocal(out=rstd_sb[:, p:p + 1], in_=stdt[:])

        # all four pairs pre-conv: VectorE takes x^2, ScalarE takes sum(x);
        # pairs 2/3 land during the AllToAll dead window, so the conv itself
        # runs with zero stats work interleaved.
        for p in range(4):
            for j in range(8):
                stats_sq_chunk(p, j, nc.vector, sqd_v)
                stats_sum_chunk_scalar(p, j)
            stats_finish(p, 8)

        # ------------------------------------------------ per-half: F build + conv
        with tc.tile_pool(name="pssml", bufs=2, space="PSUM") as psum_s, \
             tc.tile_pool(name="psconv", bufs=6, space="PSUM") as psum_c:

            def fbuild(h):
                """Gather generated kernels + build fused, rstd-scaled F."""
                pA, pB = 2 * h, 2 * h + 1
                # one strided DMA per group quadrant -> [part=c1', (tap|pwk), ci/co]
                wA = fload_p.tile([128, 10, GS], dt.bfloat16, name="wA", tag="wld")
                wB = fload_p.tile([128, 10, GS], dt.bfloat16, name="wB", tag="wld")
                for w128, g0 in ((wA, 4 * h), (wB, 4 * h + 2)):
                    nc.sync.dma_start(
                        out=w128[0:64, :, :],
                        in_=gen_out[g0, :, :].rearrange("t (c s) -> c t s", c=64),
                    )
                    nc.sync.dma_start(
                        out=w128[64:128, :, :],
                        in_=gen_out[g0 + 1, :, :].rearrange("t (c s) -> c t s", c=64),
                    )
                # fused, scaled kernels F~ : [part(ci), tap, geo, co2] bf16
                fsb = fker_p.tile([128, NTAP, 2, GS], dt.bfloat16,
                                  name=f"fsb{h}", tag=f"fsb{h}")
                for t in range(NTAP):
                    psA = psum_s.tile([128, 512], dt.float32, name="fbA", tag="s")
                    psB = psum_s.tile([128, 512], dt.float32, name="fbB", tag="s")
                    # F^T = W^T @ P^T  (fp32), per group quadrant
                    nc.tensor.matmul(out=psA[0:64, :GS], lhsT=wA[0:64, t, :],
                                     rhs=wA[0:64, NTAP, :], start=True, stop=True)
                    nc.tensor.matmul(out=psA[64:128, :GS], lhsT=wA[64:128, t, :],
                                     rhs=wA[64:128, NTAP, :], start=True, stop=True)
                    nc.tensor.matmul(out=psB[0:64, :GS], lhsT=wB[0:64, t, :],
                                     rhs=wB[0:64, NTAP, :], start=True, stop=True)
                    nc.tensor.matmul(out=psB[64:128, :GS], lhsT=wB[64:128, t, :],
                                     rhs=wB[64:128, NTAP, :], start=True, stop=True)
                    # scale by rstd (per input channel) + cast bf16
                    nc.vector.tensor_scalar(out=fsb[:, t, 0, :], in0=psA[:, :GS],
                                            scalar1=rstd_sb[:, pA:pA + 1],
                                            scalar2=None, op0=ALU.mult)
                    nc.vector.tensor_scalar(out=fsb[:, t, 1, :], in0=psB[:, :GS],
                                            scalar1=rstd_sb[:, pB:pB + 1],
                                            scalar2=None, op0=ALU.mult)
                return fsb

            def bchain(h, fsb):
                """B bias accumulation (psum holds -B); pair B quadrant-swapped."""
                pA, pB = 2 * h, 2 * h + 1
                bpsA = psum_s.tile([128, 512], dt.float32, name="bpsA", tag="s")
                bpsB = psum_s.tile([128, 512], dt.float32, name="bpsB", tag="s")
                for t in range(NTAP):
                    st_ = (t == 0)
                    sp_ = (t == NTAP - 1)
                    nc.tensor.matmul(out=bpsA[0:64, 0:1], lhsT=fsb[0:64, t, 0, :],
                                     rhs=muneg_sb[0:64, pA:pA + 1], start=st_, stop=sp_,
                                     skip_group_check=True)
                    nc.tensor.matmul(out=bpsA[64:128, 0:1], lhsT=fsb[64:128, t, 0, :],
                                     rhs=muneg_sb[64:128, pA:pA + 1], start=st_, stop=sp_,
                                     skip_group_check=True)
                    nc.tensor.matmul(out=bpsB[64:128, 0:1], lhsT=fsb[0:64, t, 1, :],
                                     rhs=muneg_sb[0:64, pB:pB + 1], start=st_, stop=sp_,
                                     skip_group_check=True)
                    nc.tensor.matmul(out=bpsB[0:64, 0:1], lhsT=fsb[64:128, t, 1, :],
                                     rhs=muneg_sb[64:128, pB:pB + 1], start=st_, stop=sp_,
                                     skip_group_check=True)
                bias_A = const_p.tile([128, 1], dt.float32, name=f"biasA{h}", tag=f"bA{h}")
                bias_B = const_p.tile([128, 1], dt.float32, name=f"biasB{h}", tag=f"bB{h}")
                nc.vector.tensor_tensor(out=bias_A[:], in0=bpsA[:, 0:1],
                                        in1=pwb_sb[:, pA:pA + 1], op=ALU.add)
                nc.vector.tensor_tensor(out=bias_B[:], in0=bpsB[:, 0:1],
                                        in1=pwb_sb[:, pB:pB + 1], op=ALU.add)
                return bias_A, bias_B

            def conv_half(h, fsb, bias_A, bias_B):
                pA, pB = 2 * h, 2 * h + 1
                imA = img[pA].rearrange("p (r c) -> p r c", c=PW)
                imB = img[pB].rearrange("p (r c) -> p r c", c=PW)
                NBLK = HW // (512 * CONV_NT)
                for blk in range(NBLK):
                    y0 = blk * 4 * CONV_NT
                    psA_ = [psum_c.tile([128, 512], dt.float32, name="cvA", tag="c")
                            for _ in range(CONV_NT)]
                    psB_ = [psum_c.tile([128, 512], dt.float32, name="cvB", tag="c")
                            for _ in range(CONV_NT)]
                    for t in range(NTAP):
                        ky, kx = t // 3, t % 3
                        st_ = (t == 0)
                        sp_ = (t == NTAP - 1)
                        for nt in range(CONV_NT):
                            y = y0 + 4 * nt
                            rA = imA[:, y + ky:y + ky + 4, kx:kx + 128]
                            rB = imB[:, y + ky:y + ky + 4, kx:kx + 128]
                            nc.tensor.matmul(out=psA_[nt][0:64, :], lhsT=fsb[0:64, t, 0, :],
                                             rhs=rA[0:64], start=st_, stop=sp_,
                                             skip_group_check=True)
                            nc.tensor.matmul(out=psA_[nt][64:128, :], lhsT=fsb[64:128, t, 0, :],
                                             rhs=rA[64:128], start=st_, stop=sp_,
                                             skip_group_check=True)
                            nc.tensor.matmul(out=psB_[nt][64:128, :], lhsT=fsb[0:64, t, 1, :],
                                             rhs=rB[0:64], start=st_, stop=sp_,
                                             skip_group_check=True)
                            nc.tensor.matmul(out=psB_[nt][0:64, :], lhsT=fsb[64:128, t, 1, :],
                                             rhs=rB[64:128], start=st_, stop=sp_,
                                             skip_group_check=True)
                    # evacuate + bias: ScalarE takes pair A, VectorE pair B;
                    # staged bf16 to halve the write-back DMA.
                    stA = stage_p.tile([128, 512 * CONV_NT], dt.bfloat16, name="stA", tag="stg")
                    stB = stage_p.tile([128, 512 * CONV_NT], dt.bfloat16, name="stB", tag="stg")
                    for nt in range(CONV_NT):
                        nc.scalar.activation(
                            out=stA[:, nt * 512:(nt + 1) * 512], in_=psA_[nt][:],
                            func=AF.Identity, bias=bias_A[:, 0:1], scale=1.0,
                        )
                        nc.vector.tensor_scalar(
                            out=stB[:, nt * 512:(nt + 1) * 512], in0=psB_[nt][:],
                            scalar1=bias_B[:, 0:1], scalar2=None, op0=ALU.add,
                        )
                    px0 = y0 * 128
                    nc.sync.dma_start(
                        out=out_dev[pA * 128:(pA + 1) * 128, px0:px0 + 512 * CONV_NT],
                        in_=stA[:],
                    )
                    nc.sync.dma_start(
                        out=out_dev[pB * 128:(pB + 1) * 128, px0:px0 + 512 * CONV_NT],
                        in_=stB[:],
                    )

            fsb0 = fbuild(0)
            bias_A0, bias_B0 = bchain(0, fsb0)
            fsb1 = fbuild(1)
            bias_A1, bias_B1 = bchain(1, fsb1)
            conv_half(0, fsb0, bias_A0, bias_B0)
            conv_half(1, fsb1, bias_A1, bias_B1)

    nc.compile()
    return nc


_NC_CACHE = {}


def kernel(**inputs) -> np.ndarray:
    from concourse.bass_utils import run_bass_kernel_spmd

    in_maps = _host_prep(**inputs)
    if "nc" not in _NC_CACHE:
        _NC_CACHE["nc"] = build_nc()
    nc = _NC_CACHE["nc"]
    res = run_bass_kernel_spmd(nc, in_maps, core_ids=list(range(N)))
    return _unshard(res.results)


if __name__ == "__main__":
    import jax

    import reference

    with jax.default_device(jax.devices("cpu")[0]):
        inputs = {k: np.asarray(v) for k, v in reference.setup_inputs().items()}
        expected = np.asarray(reference.reference(**inputs))
    actual = kernel(**inputs)
    err = np.sqrt(((actual - expected) ** 2).mean()) / np.sqrt((expected ** 2).mean())
    print("Relative error:", err)


# revision 54
# speedup vs baseline: 1.0983x; 1.0009x over previous
"""AdaConv (nn_AdaConv_46445776339355) — 8-core TRN2 Bass kernel.

Strategy
--------
Data-parallel over batch N=8: core n owns sample n end-to-end for the heavy
instance-norm + grouped-conv work.  The kernel *generator* (dw_w is 256 MiB)
is tensor-parallel: core j holds the output-channel shard j of dw_w / pwk_w,
computes the generated kernels for ALL samples on its shard, and an AllToAll
routes each sample's kernels to its owning core.

Algebraic fusions (all computed on device):
  * pointwise o depthwise = one fused per-group kernel  F_t = P @ W_t
  * instance norm folded into the fused kernels:
        y = sum_t F_t @ ((x-mu)/sigma)_pad = sum_t (F_t/sigma_ci) @ x_pad - B
    with B = sum_t (F_t/sigma) @ mu  (position independent, reflect-pad safe)
  * biases (dw_b, pwk_b, pwb_b) folded in via K=1 matmul rows.

The grouped conv (8 groups of 64->64 ch, 3x3) runs as 4 concurrent 64x64
matmuls in the 4 PE-array quadrants (tile_position packing) => full 128x128
PE utilization, bf16, 9 shifted-AP taps accumulating in PSUM.

v2 flow restructure (from trace analysis of the 455us baseline):
  * generator weight chunks stream first on the sync/scalar DMA queues with
    nothing queued ahead of them (the 21 MiB stream is the serial critical
    path: ~60us at full BW); pwk chunk DMAs are hoisted before the PSUM
    evacuation copies so the stream never stalls behind compute waits.
  * img (predicted) loads are deferred behind the weight stream via a
    dependency stub on a late weight chunk, in 2 half-tile DMAs each so the
    stats reductions overlap arrival.
  * instance-norm stats for pairs 0/1 run on VectorE during the generator
    phase; pairs 2/3 run on GpSimdE during conv h=0 (VectorE evicts PSUM).
  * PSUM eviction split: ScalarE takes pair A, VectorE pair B; output staged
    bf16 (halves write-back DMA; rel-err budget is loose).
  * F-build gathers collapsed to 4 strided DMAs per half.
"""

import sys
import numpy as np

sys.path.insert(0, "/opt/trn_rl_repo")

import ml_dtypes

BF16 = ml_dtypes.bfloat16

# ---------------- problem constants (hardcoded per the harness contract) ----
N = 8            # batch == number of cores
C = 512          # channels
H = W = 128
HW = H * W       # 16384
PW = W + 2       # 130 padded
PA = PW * PW     # 16900
PAH = 65 * PW    # 8450: half-tile split (pad rows 0-64 | 65-129)
SD = 512         # style dim
NG = 8           # groups
GS = 64          # group size (channels per group)
KDW = SD * 4     # 2048 contraction dim of the dw generator
OSH = 4096       # dw/pwk output-channel shard per core (32768/8); == one group block
NTAP = 9
EPS = 1e-5
VAR_CORR = float(HW) / float(HW - 1)  # ddof=1 correction

# device output channel order: per pair of groups (2h, 2h+1) natural, the odd
# pairs (pB) have their two 64-blocks swapped (quadrant output packing).
TAU_BLOCK = [0, 1, 3, 2, 4, 5, 7, 6]  # true 64-block of device 64-block d


def _host_prep(style_encoding, predicted, dw_w, dw_b, pwk_w, pwk_b, pwb_w, pwb_b):
    """Pure data-movement / dtype-cast host prep. Returns per-core input maps."""
    f32 = np.float32
    se = np.asarray(style_encoding, f32)
    pred = np.asarray(predicted, f32)

    # --- patches for the dw generator conv: reflect pad 1, 2x2 windows s=2 ---
    sep = np.pad(se, ((0, 0), (0, 0), (1, 1), (1, 1)), mode="reflect")  # (8,512,6,6)
    blocks = sep.reshape(N, SD, 3, 2, 3, 2)  # [n,c,oy,ky,ox,kx]
    patches = np.ascontiguousarray(
        blocks.transpose(1, 3, 5, 0, 2, 4).reshape(KDW, N * NTAP)
    ).astype(BF16)  # [(c,ky,kx), (n,oy,ox)] = [2048, 72]
    # pre-tiled for a contiguous SBUF DMA: [128, kt, m]
    patches_t = np.ascontiguousarray(
        patches.reshape(16, 128, N * NTAP).transpose(1, 0, 2).reshape(128, 16 * N * NTAP)
    )

    # --- dw generator weights, transposed + sharded on output channels ---
    dwt_full = np.ascontiguousarray(dw_w.reshape(C * GS, KDW).T).astype(BF16)  # [2048, 32768]
    dwb_full = np.asarray(dw_b, f32).reshape(1, C * GS).astype(BF16)

    # --- pwk: permute columns to (g, cm, co2) so the gathered row IS P^T ---
    pwk_t = np.asarray(pwk_w, f32).reshape(NG, GS, GS, SD)  # [g, co2, cm, sd]
    pwkt_full = np.ascontiguousarray(
        pwk_t.transpose(3, 0, 2, 1).reshape(SD, C * GS)
    ).astype(BF16)  # [sd, (g, cm, co2)]
    pwkb_full = (
        np.asarray(pwk_b, f32).reshape(NG, GS, GS).transpose(0, 2, 1).reshape(1, C * GS)
    ).astype(BF16)

    # --- pwb: transposed, columns in DEVICE channel order tau ---
    tau_rows = np.concatenate([np.arange(GS) + t * GS for t in TAU_BLOCK])  # [512]
    pwbt = np.ascontiguousarray(np.asarray(pwb_w, f32)[tau_rows, :].T).astype(BF16)  # [sd, out_dev]
    pwbt_t = np.ascontiguousarray(
        pwbt.reshape(4, 128, C).transpose(1, 0, 2).reshape(128, 4 * C))
    pwbb = np.asarray(pwb_b, f32)[tau_rows].reshape(1, C).astype(BF16)

    # --- styleT for sd computation on device: pre-tiled [128, kt, (n, px)] f32 ---
    styleT = np.ascontiguousarray(se.transpose(1, 0, 2, 3).reshape(SD, N * 16)).astype(f32)
    styleT_t = np.ascontiguousarray(
        styleT.reshape(4, 128, N * 16).transpose(1, 0, 2).reshape(128, 4 * N * 16))

    in_maps = []
    for j in range(N):
        pp = np.pad(pred[j], ((0, 0), (1, 1), (1, 1)), mode="reflect").reshape(C, PA)
        sel = np.zeros((128, N), f32)
        sel[:, j] = 1.0
        in_maps.append(
            dict(
                pred_pad=np.ascontiguousarray(pp).astype(BF16),
                patches=patches_t,
                styleT=styleT_t,
                sel=sel,
                dwt=np.ascontiguousarray(dwt_full[:, j * OSH:(j + 1) * OSH]),
                dwb=np.ascontiguousarray(dwb_full[:, j * OSH:(j + 1) * OSH]),
                pwkt=np.ascontiguousarray(pwkt_full[:, j * OSH:(j + 1) * OSH]),
                pwkb=np.ascontiguousarray(pwkb_full[:, j * OSH:(j + 1) * OSH]),
                pwbt=pwbt_t,
                pwbb=pwbb,
            )
        )
    return in_maps


def _unshard(results):
    """results[j]['out'] is [512, 16384] bf16 in device channel order."""
    out = np.empty((N, C, H, W), np.float32)
    for j in range(N):
        dev = np.asarray(results[j]["out"], np.float32).reshape(C, H, W)
        for d, t in enumerate(TAU_BLOCK):
            out[j, t * GS:(t + 1) * GS] = dev[d * GS:(d + 1) * GS]
    return out


# how many 512-px N-tiles are accumulated per psum set before evacuation
CONV_NT = 2


def build_nc():
    from concourse import bacc, mybir, tile
    from contextlib import ExitStack

    dt = mybir.dt
    AF = mybir.ActivationFunctionType
    ALU = mybir.AluOpType

    nc = bacc.Bacc(num_devices=N)

    pred_pad = nc.declare_dram_parameter("pred_pad", [C, PA], dt.bfloat16, isOutput=False)
    patches = nc.declare_dram_parameter("patches", [128, 16 * N * NTAP], dt.bfloat16, isOutput=False)
    styleT = nc.declare_dram_parameter("styleT", [128, 4 * N * 16], dt.float32, isOutput=False)
    sel = nc.declare_dram_parameter("sel", [128, N], dt.float32, isOutput=False)
    dwt = nc.declare_dram_parameter("dwt", [KDW, OSH], dt.bfloat16, isOutput=False)
    dwb = nc.declare_dram_parameter("dwb", [1, OSH], dt.bfloat16, isOutput=False)
    pwkt = nc.declare_dram_parameter("pwkt", [SD, OSH], dt.bfloat16, isOutput=False)
    pwkb = nc.declare_dram_parameter("pwkb", [1, OSH], dt.bfloat16, isOutput=False)
    pwbt = nc.declare_dram_parameter("pwbt", [128, 4 * C], dt.bfloat16, isOutput=False)
    pwbb = nc.declare_dram_parameter("pwbb", [1, C], dt.bfloat16, isOutput=False)
    out_dev = nc.declare_dram_parameter("out", [C, HW], dt.bfloat16, isOutput=True)

    replica = [list(range(N))]

    with tile.TileContext(nc) as tc, ExitStack() as ctx:
        dram = ctx.enter_context(tc.tile_pool(name="dram", bufs=1, space="DRAM"))
        gen_all = dram.tile([N, 10, OSH], dt.bfloat16, tag="gall")
        gen_out = dram.tile([N, 10, OSH], dt.bfloat16, tag="gout")

        const_p = ctx.enter_context(tc.tile_pool(name="const", bufs=1))
        pred_p = ctx.enter_context(tc.tile_pool(name="pred", bufs=1))
        stats_p = ctx.enter_context(tc.tile_pool(name="stats", bufs=2))
        fker_p = ctx.enter_context(tc.tile_pool(name="fker", bufs=1))
        fload_p = ctx.enter_context(tc.tile_pool(name="fload", bufs=2))
        stage_p = ctx.enter_context(tc.tile_pool(name="stage", bufs=2))
        wstream_p = ctx.enter_context(tc.tile_pool(name="wstream", bufs=6))
        gstg_p = ctx.enter_context(tc.tile_pool(name="gstg", bufs=2))

        # ------------------------------------------------ tiny const loads (gpsimd)
        pt = const_p.tile([128, 16, N * NTAP], dt.bfloat16, tag="pt")
        nc.gpsimd.dma_start(out=pt[:], in_=patches.rearrange("p (kt m) -> p kt m", kt=16))

        st = const_p.tile([128, 4, N * 16], dt.float32, tag="st")
        nc.gpsimd.dma_start(out=st[:], in_=styleT.rearrange("p (kt m) -> p kt m", kt=4))

        sel_sb = const_p.tile([128, N], dt.float32, tag="sel")
        nc.gpsimd.dma_start(out=sel_sb[:], in_=sel[:, :])

        pwbt_sb = const_p.tile([128, 4, C], dt.bfloat16, tag="pwbt")
        nc.gpsimd.dma_start(out=pwbt_sb[:], in_=pwbt.rearrange("p (kt m) -> p kt m", kt=4))

        pwbb_sb = const_p.tile([1, C], dt.bfloat16, tag="pwbb")
        nc.gpsimd.dma_start(out=pwbb_sb[:], in_=pwbb[:, :])

        ones = const_p.tile([1, 128], dt.bfloat16, tag="ones")
        nc.vector.memset(ones[:], 1.0)

        # ------------------------------------------------ sd = mean(style, px)
        # (VectorE, tiny; ready long before the pwk generator needs it)
        sdf = const_p.tile([128, 4, N], dt.float32, tag="sdf")
        sdb = const_p.tile([128, 4, N], dt.bfloat16, tag="sdb")
        sdnb = const_p.tile([128, 4], dt.bfloat16, tag="sdnb")  # own-sample column
        tmp8 = stats_p.tile([128, N], dt.float32, tag="tmp8")
        sdn_f = const_p.tile([128, 4], dt.float32, tag="sdnf")
        for kt in range(4):
            nc.vector.tensor_reduce(
                out=sdf[:, kt, :],
                in_=st[:, kt, :].rearrange("p (n x) -> p n x", x=16),
                axis=mybir.AxisListType.X,
                op=ALU.add,
            )
            nc.vector.tensor_scalar(
                out=sdb[:, kt, :], in0=sdf[:, kt, :], scalar1=1.0 / 16.0,
                scalar2=None, op0=ALU.mult,
            )
            # own sample's sd column (via one-hot sel): sdn = sum_n sdf[:,n]*sel[:,n]
            nc.vector.tensor_tensor(
                out=tmp8[:], in0=sdf[:, kt, :], in1=sel_sb[:], op=ALU.mult
            )
            nc.vector.tensor_reduce(
                out=sdn_f[:, kt:kt + 1], in_=tmp8[:], axis=mybir.AxisListType.X, op=ALU.add
            )
        nc.vector.tensor_scalar(
            out=sdnb[:], in0=sdn_f[:], scalar1=1.0 / 16.0, scalar2=None, op0=ALU.mult
        )

        # ------------------------------------------------ generator phase (PE)
        # 0.5 MiB weight chunks round-robined over the 3 DMA-capable engine
        # queues with 8 wstream buffers: each queue keeps ~2.7 chunks in
        # flight so the HBM stream never stalls on matmul consumption.
        dma_engs = [nc.sync, nc.scalar, nc.gpsimd]
        wt_tiles = []
        HOSH = OSH // 2  # 2048 columns per chunk
        with tc.tile_pool(name="psgen", bufs=8, space="PSUM") as psum_g:
            # bias rows preloaded at t=0 (tiny) so the dw accumulation stop
            # never waits on a late queue position
            biasrow = gstg_p.tile([1, OSH], dt.bfloat16, name="biasrow", tag="br", bufs=1)
            nc.sync.dma_start(out=biasrow[0:1, :], in_=dwb[0:1, :])
            biasrow2 = gstg_p.tile([1, OSH], dt.bfloat16, name="biasrow2", tag="br2", bufs=1)
            nc.scalar.dma_start(out=biasrow2[0:1, :], in_=pwkb[0:1, :])
            # dw generator: 32 weight chunks, 8 psum blocks
            ps_dw = [psum_g.tile([128, 512], dt.float32, name=f"dwg{b}", tag="g")
                     for b in range(8)]
            ci = 0
            for kt in range(16):
                for hf in range(2):
                    wt = wstream_p.tile([128, HOSH], dt.bfloat16, name="wt", tag="w")
                    dma_engs[ci % 3].dma_start(
                        out=wt[:], in_=dwt[kt * 128:(kt + 1) * 128, hf * HOSH:(hf + 1) * HOSH])
                    wt_tiles.append(wt)
                    ci += 1
                    for bb in range(4):
                        b = 4 * hf + bb
                        nc.tensor.matmul(
                            out=ps_dw[b][:N * NTAP, :],
                            lhsT=pt[:, kt, :],
                            rhs=wt[:, bb * 512:(bb + 1) * 512],
                            start=(kt == 0), stop=False,
                        )
            # pwk chunks follow immediately on the same rotation.  The pwk
            # accumulators live in rows 96:104 of the SAME psum banks as the
            # dw blocks (tile_position col 96) so pwk matmuls never wait on
            # bank releases -> no circular buffer/bank stalls.
            pwk_tiles = []
            for kt in range(4):
                for hf in range(2):
                    wt = wstream_p.tile([128, HOSH], dt.bfloat16, name="wt", tag="w")
                    dma_engs[ci % 3].dma_start(
                        out=wt[:], in_=pwkt[kt * 128:(kt + 1) * 128, hf * HOSH:(hf + 1) * HOSH])
                    wt_tiles.append(wt)
                    pwk_tiles.append(wt)
                    ci += 1
                    for bb in range(4):
                        b = 4 * hf + bb
                        nc.tensor.matmul(
                            out=ps_dw[b][96:96 + N, :],
                            lhsT=sdb[:, kt, :],
                            rhs=wt[:, bb * 512:(bb + 1) * 512],
                            start=(kt == 0), stop=False,
                            skip_group_check=True,
                            tile_position=(0, 96),
                        )

            # img deferral: write one corner element of each img half-range
            # from a late weight chunk.  The img DMA then has a WAW dependency
            # on the stub, and the stub a RAW dependency on the chunk -> the
            # 17.3 MiB of img traffic cannot start until the critical weight
            # stream is nearly done.  (A plain ordering stub doesn't work: the
            # Tile scheduler is dependency-driven, not program-ordered.)
            img = [pred_p.tile([128, PA], dt.bfloat16, name=f"img{p}", tag=f"img{p}")
                   for p in range(4)]
            for p in range(4):
                for hh in range(2):
                    nc.gpsimd.tensor_scalar(
                        out=img[p][0:1, hh * PAH:hh * PAH + 1],
                        in0=wt_tiles[28][0:1, 0:1], scalar1=1.0,
                        scalar2=None, op0=ALU.mult,
                    )
            # img loads on sync/gpsimd only: the Scalar queue must stay clear
            # for the generator psum evacuation copies.
            img_engs = [nc.sync, nc.gpsimd]
            for i, (p, hh) in enumerate((p, hh) for p in range(4) for hh in range(2)):
                img_engs[i % 2].dma_start(
                    out=img[p][:, hh * PAH:hh * PAH + PAH],
                    in_=pred_pad[p * 128:(p + 1) * 128, hh * PAH:hh * PAH + PAH])

            # dw bias rows + psum evacuation + store to gen_all
            for b in range(8):
                nc.tensor.matmul(
                    out=ps_dw[b][:N * NTAP, :],
                    lhsT=ones[:1, :N * NTAP],
                    rhs=biasrow[0:1, b * 512:(b + 1) * 512],
                    start=False, stop=True,
                )
                gsb = gstg_p.tile([N * NTAP, 512], dt.bfloat16, name="gsb", tag="gs")
                nc.vector.tensor_copy(out=gsb[:], in_=ps_dw[b][:N * NTAP, :])
                nc.sync.dma_start(
                    out=gen_all[:, 0:NTAP, b * 512:(b + 1) * 512],
                    in_=gsb[:, :],
                )

            # pwk bias + evacuation
            g2sb_last = None
            for b in range(8):
                nc.tensor.matmul(
                    out=ps_dw[b][96:96 + N, :],
                    lhsT=ones[:1, :N],
                    rhs=biasrow2[0:1, b * 512:(b + 1) * 512],
                    start=False, stop=True,
                    skip_group_check=True,
                    tile_position=(0, 96),
                )
                g2sb = gstg_p.tile([N, 512], dt.bfloat16, name="g2sb", tag="gs2")
                nc.vector.tensor_copy(out=g2sb[:], in_=ps_dw[b][96:96 + N, :])
                nc.sync.dma_start(
                    out=gen_all[:, NTAP, b * 512:(b + 1) * 512], in_=g2sb[:]
                )
                g2sb_last = g2sb

            # pwb bias chain (device channel order)
            pwb_sb = const_p.tile([128, 4], dt.float32, tag="pwbv")
            for m in range(4):
                pm = psum_g.tile([128, 512], dt.float32, name="pwbps", tag="g")
                for kt in range(4):
                    nc.tensor.matmul(
                        out=pm[:, 0:1],
                        lhsT=pwbt_sb[:, kt, m * 128:(m + 1) * 128],
                        rhs=sdnb[:, kt:kt + 1],
                        start=(kt == 0), stop=False,
                    )
                nc.tensor.matmul(
                    out=pm[:, 0:1],
                    lhsT=pwbb_sb[:1, m * 128:(m + 1) * 128],
                    rhs=ones[:1, 0:1],
                    start=False, stop=True,
                )
                nc.vector.tensor_copy(out=pwb_sb[:, m:m + 1], in_=pm[:, 0:1])

            nc.gpsimd.collective_compute(
                "AllToAll",
                ALU.bypass,
                replica_groups=replica,
                ins=[gen_all[:, :, :].opt()],
                outs=[gen_out[:, :, :].opt()],
            )

        # ------------------------------------------------ instance-norm stats
        # var = E[x^2] - mu^2 (+ ddof correction).  Sum-of-squares via STT with
        # per-chunk accum, plain sum via tensor_reduce, both split at the img
        # half-DMA boundary (pad rows 0-64 | 65-129) so they overlap arrival.
        rstd_sb = const_p.tile([128, 4], dt.float32, tag="rstd")
        muneg_sb = const_p.tile([128, 4], dt.bfloat16, tag="muneg")

        # per-pair accumulators (pairs 2/3 fill theirs mid-conv)
        acc8 = {p: stats_p.tile([128, 8], dt.float32, name=f"acc8_{p}", tag=f"acc8_{p}")
                for p in range(4)}
        ssumc = {p: stats_p.tile([128, 8], dt.float32, name=f"ssum_{p}", tag=f"ssum_{p}")
                 for p in range(4)}
        sqd_v = stats_p.tile([128, 16, 128], dt.bfloat16, name="sqdv", tag="sqdv", bufs=1)
        sqd_s = stats_p.tile([128, 16, 128], dt.bfloat16, name="sqds", tag="sqds", bufs=1)

        def stats_sq_chunk(p, j, eng, trash):
            """accumulate sum(x^2) of 16-row chunk j into acc8[p][:, j]."""
            view = img[p].rearrange("p (r c) -> p r c", c=PW)
            xs = view[:, 1 + 16 * j:1 + 16 * (j + 1), 1:129]
            if eng is nc.vector:
                eng.scalar_tensor_tensor(
                    out=trash[:], in0=xs, scalar=1.0, in1=xs,
                    op0=ALU.mult, op1=ALU.mult,
                    accum_out=acc8[p][:, j:j + 1],
                )
            else:
                eng.activation(
                    out=trash[:], in_=xs, func=AF.Square,
                    accum_out=acc8[p][:, j:j + 1],
                )

        def stats_sum_chunk_scalar(p, j):
            """sum(x) of 16-row chunk j into ssumc[p][:, j] (ScalarE, Copy
            activation with accum; trash via sqd_s so the sqd_s gate applies)."""
            view = img[p].rearrange("p (r c) -> p r c", c=PW)
            xs = view[:, 1 + 16 * j:1 + 16 * (j + 1), 1:129]
            nc.scalar.activation(
                out=sqd_s[:], in_=xs, func=AF.Copy,
                accum_out=ssumc[p][:, j:j + 1],
            )

        def stats_finish(p, nsum):
            """tiny finals: VectorE + one ScalarE sqrt (gated behind the
            generator evacuations so it can't head-block the Scalar queue)."""
            ssq = stats_p.tile([128, 1], dt.float32, name=f"ssq{p}", tag=f"ssq{p}")
            nc.vector.tensor_reduce(
                out=ssq[:], in_=acc8[p][:], axis=mybir.AxisListType.X, op=ALU.add
            )
            ssum = stats_p.tile([128, 1], dt.float32, name=f"ssm{p}", tag=f"ssm{p}")
            nc.vector.tensor_reduce(
                out=ssum[:], in_=ssumc[p][:, 0:nsum], axis=mybir.AxisListType.X, op=ALU.add
            )
            mu = stats_p.tile([128, 1], dt.float32, name=f"mu{p}", tag=f"mu{p}")
            nc.vector.tensor_scalar(
                out=mu[:], in0=ssum[:], scalar1=1.0 / HW, scalar2=None, op0=ALU.mult
            )
            nc.vector.tensor_scalar(
                out=muneg_sb[:, p:p + 1], in0=mu[:], scalar1=-1.0,
                scalar2=None, op0=ALU.mult,
            )
            ex2 = stats_p.tile([128, 1], dt.float32, name=f"ex2_{p}", tag=f"ex2_{p}")
            nc.vector.tensor_scalar(
                out=ex2[:], in0=ssq[:], scalar1=1.0 / HW, scalar2=None, op0=ALU.mult
            )
            mu2 = stats_p.tile([128, 1], dt.float32, name=f"mu2_{p}", tag=f"mu2_{p}")
            nc.vector.tensor_tensor(out=mu2[:], in0=mu[:], in1=mu[:], op=ALU.mult)
            varp = stats_p.tile([128, 1], dt.float32, name=f"varp{p}", tag=f"varp{p}")
            nc.vector.tensor_tensor(out=varp[:], in0=ex2[:], in1=mu2[:], op=ALU.subtract)
            vtmp = stats_p.tile([128, 1], dt.float32, name=f"vt{p}", tag=f"vt{p}")
            nc.vector.tensor_scalar(
                out=vtmp[:], in0=varp[:], scalar1=VAR_CORR, scalar2=EPS,
                op0=ALU.mult, op1=ALU.add,
            )
            stdt = stats_p.tile([128, 1], dt.float32, name=f"st{p}", tag=f"st{p}")
            nc.scalar.sqrt(stdt[:], vtmp[:])
            nc.vector.reciprocal(out=rstd_sb[:, p:p + 1], in_=stdt[:])

        # pairs 0/1 pre-conv: VectorE takes x^2, ScalarE takes sum(x); the
        # evacuation copies (VectorE) slot in right behind the 16 sq chunks.
        # pairs 2/3 run interleaved with conv h=0 (below).
        for p in (0, 1):
            for j in range(8):
                stats_sq_chunk(p, j, nc.vector, sqd_v)
                stats_sum_chunk_scalar(p, j)
            stats_finish(p, 8)

        # ------------------------------------------------ per-half: F build + conv
        with tc.tile_pool(name="pssml", bufs=2, space="PSUM") as psum_s, \
             tc.tile_pool(name="psconv", bufs=6, space="PSUM") as psum_c:

            def fbuild(h):
                """Gather generated kernels + build fused, rstd-scaled F."""
                pA, pB = 2 * h, 2 * h + 1
                # one strided DMA per group quadrant -> [part=c1', (tap|pwk), ci/co]
                wA = fload_p.tile([128, 10, GS], dt.bfloat16, name="wA", tag="wld")
                wB = fload_p.tile([128, 10, GS], dt.bfloat16, name="wB", tag="wld")
                for w128, g0 in ((wA, 4 * h), (wB, 4 * h + 2)):
                    nc.sync.dma_start(
                        out=w128[0:64, :, :],
                        in_=gen_out[g0, :, :].rearrange("t (c s) -> c t s", c=64),
                    )
                    nc.sync.dma_start(
                        out=w128[64:128, :, :],
                        in_=gen_out[g0 + 1, :, :].rearrange("t (c s) -> c t s", c=64),
                    )
                # fused, scaled kernels F~ : [part(ci), tap, geo, co2] bf16
                fsb = fker_p.tile([128, NTAP, 2, GS], dt.bfloat16,
                                  name=f"fsb{h}", tag=f"fsb{h}")
                for t in range(NTAP):
                    psA = psum_s.tile([128, 512], dt.float32, name="fbA", tag="s")
                    psB = psum_s.tile([128, 512], dt.float32, name="fbB", tag="s")
                    # F^T = W^T @ P^T  (fp32), per group quadrant
                    nc.tensor.matmul(out=psA[0:64, :GS], lhsT=wA[0:64, t, :],
                                     rhs=wA[0:64, NTAP, :], start=True, stop=True)
                    nc.tensor.matmul(out=psA[64:128, :GS], lhsT=wA[64:128, t, :],
                                     rhs=wA[64:128, NTAP, :], start=True, stop=True)
                    nc.tensor.matmul(out=psB[0:64, :GS], lhsT=wB[0:64, t, :],
                                     rhs=wB[0:64, NTAP, :], start=True, stop=True)
                    nc.tensor.matmul(out=psB[64:128, :GS], lhsT=wB[64:128, t, :],
                                     rhs=wB[64:128, NTAP, :], start=True, stop=True)
                    # scale by rstd (per input channel) + cast bf16
                    nc.vector.tensor_scalar(out=fsb[:, t, 0, :], in0=psA[:, :GS],
                                            scalar1=rstd_sb[:, pA:pA + 1],
                                            scalar2=None, op0=ALU.mult)
                    nc.vector.tensor_scalar(out=fsb[:, t, 1, :], in0=psB[:, :GS],
                                            scalar1=rstd_sb[:, pB:pB + 1],
                                            scalar2=None, op0=ALU.mult)
                return fsb

            def bchain(h, fsb):
                """B bias accumulation (psum holds -B); pair B quadrant-swapped."""
                pA, pB = 2 * h, 2 * h + 1
                bpsA = psum_s.tile([128, 512], dt.float32, name="bpsA", tag="s")
                bpsB = psum_s.tile([128, 512], dt.float32, name="bpsB", tag="s")
                for t in range(NTAP):
                    st_ = (t == 0)
                    sp_ = (t == NTAP - 1)
                    nc.tensor.matmul(out=bpsA[0:64, 0:1], lhsT=fsb[0:64, t, 0, :],
                                     rhs=muneg_sb[0:64, pA:pA + 1], start=st_, stop=sp_,
                                     skip_group_check=True)
                    nc.tensor.matmul(out=bpsA[64:128, 0:1], lhsT=fsb[64:128, t, 0, :],
                                     rhs=muneg_sb[64:128, pA:pA + 1], start=st_, stop=sp_,
                                     skip_group_check=True)
                    nc.tensor.matmul(out=bpsB[64:128, 0:1], lhsT=fsb[0:64, t, 1, :],
                                     rhs=muneg_sb[0:64, pB:pB + 1], start=st_, stop=sp_,
                                     skip_group_check=True)
                    nc.tensor.matmul(out=bpsB[0:64, 0:1], lhsT=fsb[64:128, t, 1, :],
                                     rhs=muneg_sb[64:128, pB:pB + 1], start=st_, stop=sp_,
                                     skip_group_check=True)
                bias_A = const_p.tile([128, 1], dt.float32, name=f"biasA{h}", tag=f"bA{h}")
                bias_B = const_p.tile([128, 1], dt.float32, name=f"biasB{h}", tag=f"bB{h}")
                nc.vector.tensor_tensor(out=bias_A[:], in0=bpsA[:, 0:1],
                                        in1=pwb_sb[:, pA:pA + 1], op=ALU.add)
                nc.vector.tensor_tensor(out=bias_B[:], in0=bpsB[:, 0:1],
                                        in1=pwb_sb[:, pB:pB + 1], op=ALU.add)
                return bias_A, bias_B

            def conv_half(h, fsb, bias_A, bias_B):
                pA, pB = 2 * h, 2 * h + 1
                imA = img[pA].rearrange("p (r c) -> p r c", c=PW)
                imB = img[pB].rearrange("p (r c) -> p r c", c=PW)
                NBLK = HW // (512 * CONV_NT)
                for blk in range(NBLK):
                    y0 = blk * 4 * CONV_NT
                    psA_ = [psum_c.tile([128, 512], dt.float32, name="cvA", tag="c")
                            for _ in range(CONV_NT)]
                    psB_ = [psum_c.tile([128, 512], dt.float32, name="cvB", tag="c")
                            for _ in range(CONV_NT)]
                    for t in range(NTAP):
                        ky, kx = t // 3, t % 3
                        st_ = (t == 0)
                        sp_ = (t == NTAP - 1)
                        for nt in range(CONV_NT):
                            y = y0 + 4 * nt
                            rA = imA[:, y + ky:y + ky + 4, kx:kx + 128]
                            rB = imB[:, y + ky:y + ky + 4, kx:kx + 128]
                            nc.tensor.matmul(out=psA_[nt][0:64, :], lhsT=fsb[0:64, t, 0, :],
                                             rhs=rA[0:64], start=st_, stop=sp_,
                                             skip_group_check=True)
                            nc.tensor.matmul(out=psA_[nt][64:128, :], lhsT=fsb[64:128, t, 0, :],
                                             rhs=rA[64:128], start=st_, stop=sp_,
                                             skip_group_check=True)
                            nc.tensor.matmul(out=psB_[nt][64:128, :], lhsT=fsb[0:64, t, 1, :],
                                             rhs=rB[0:64], start=st_, stop=sp_,
                                             skip_group_check=True)
                            nc.tensor.matmul(out=psB_[nt][0:64, :], lhsT=fsb[64:128, t, 1, :],
                                             rhs=rB[64:128], start=st_, stop=sp_,
                                             skip_group_check=True)
                    # evacuate + bias: ScalarE takes pair A, VectorE pair B;
                    # staged bf16 to halve the write-back DMA.
                    stA = stage_p.tile([128, 512 * CONV_NT], dt.bfloat16, name="stA", tag="stg")
                    stB = stage_p.tile([128, 512 * CONV_NT], dt.bfloat16, name="stB", tag="stg")
                    for nt in range(CONV_NT):
                        nc.scalar.activation(
                            out=stA[:, nt * 512:(nt + 1) * 512], in_=psA_[nt][:],
                            func=AF.Identity, bias=bias_A[:, 0:1], scale=1.0,
                        )
                        nc.vector.tensor_scalar(
                            out=stB[:, nt * 512:(nt + 1) * 512], in0=psB_[nt][:],
                            scalar1=bias_B[:, 0:1], scalar2=None, op0=ALU.add,
                        )
                    px0 = y0 * 128
                    nc.sync.dma_start(
                        out=out_dev[pA * 128:(pA + 1) * 128, px0:px0 + 512 * CONV_NT],
                        in_=stA[:],
                    )
                    nc.sync.dma_start(
                        out=out_dev[pB * 128:(pB + 1) * 128, px0:px0 + 512 * CONV_NT],
                        in_=stB[:],
                    )

                    if h == 0:
                        # pairs 2/3 stats interleaved with conv h=0 (VectorE
                        # x^2 chunks + ScalarE sums slot between eviction
                        # batches), then F-build h=1 in the tail blocks so the
                        # h0->h1 transition has no PE bubble.
                        if 2 <= blk < 10:
                            p23 = 2 + (blk - 2) // 4
                            j0 = 2 * ((blk - 2) % 4)
                            stats_sq_chunk(p23, j0, nc.vector, sqd_v)
                            stats_sq_chunk(p23, j0 + 1, nc.vector, sqd_v)
                            stats_sum_chunk_scalar(p23, j0)
                            stats_sum_chunk_scalar(p23, j0 + 1)
                        elif blk == 10:
                            stats_finish(2, 8)
                        elif blk == 11:
                            stats_finish(3, 8)
                        elif blk == 13:
                            h1_state["fsb"] = fbuild(1)
                        elif blk == 15:
                            h1_state["bias"] = bchain(1, h1_state["fsb"])

            h1_state = {}
            fsb0 = fbuild(0)
            bias_A0, bias_B0 = bchain(0, fsb0)
            conv_half(0, fsb0, bias_A0, bias_B0)
            conv_half(1, h1_state["fsb"], *h1_state["bias"])

    nc.compile()
    return nc


_NC_CACHE = {}


def kernel(**inputs) -> np.ndarray:
    from concourse.bass_utils import run_bass_kernel_spmd

    in_maps = _host_prep(**inputs)
    if "nc" not in _NC_CACHE:
        _NC_CACHE["nc"] = build_nc()
    nc = _NC_CACHE["nc"]
    res = run_bass_kernel_spmd(nc, in_maps, core_ids=list(range(N)))
    return _unshard(res.results)


if __name__ == "__main__":
    import jax

    import reference

    with jax.default_device(jax.devices("cpu")[0]):
        inputs = {k: np.asarray(v) for k, v in reference.setup_inputs().items()}
        expected = np.asarray(reference.reference(**inputs))
    actual = kernel(**inputs)
    err = np.sqrt(((actual - expected) ** 2).mean()) / np.sqrt((expected ** 2).mean())
    print("Relative error:", err)
